# revision 9
# baseline (speedup 1.0000x reference)
"""Trainium2 Bass kernel for a 2-layer GAT (PyG GATConv, concat heads) +
global mean pool + linear head, distributed over 8 NeuronCores.

Strategy (self-contained; shapes hardcoded for this problem):
  - Destination-shard nodes across the 8 cores (2500 each); a core owns the
    edges whose destination lies in its slice. Graph pooling is shard-local.
  - he = edge_attr @ We is never materialized: its only use is
    a_e = he . att_e, which collapses to a_e = edge_attr @ we with the [4, 8]
    matrix we[d, h] = We[d, h*64:(h+1)*64] @ att_e[h] (host weight folding).
  - Self-loops (fill_value='mean') are folded analytically: the self-loop's
    a_e equals segment_sum(a_e)/max(deg, 1); both stats are accumulated as
    extra columns of the edge-aggregation matmul.
  - Softmax without max-subtraction: out_i = sum_e exp(r_e) h_src / sum exp(r_e)
    is mathematically identical to the max-normalized form (logits are O(10)).
  - Edge aggregation per 128-destination tile = PE matmul with the one-hot
    scatter matrix C[e, d] = (dst_local[e] == d), built on DVE from an iota
    constant; messages are dma_gather'ed source rows scaled by exp-logits.
  - Two SPMD launches: launch 1 builds the layer-1 node table x @ W1ext
    (features + a_s + a_d columns), runs the layer-1 edge phase, and emits
    each core's slice of the layer-2 table h2 = relu(out1) @ W2ext. The host
    concatenates slices (pure data movement) and launch 2 runs the layer-2
    edge phase + pooling + linear head.

Graded entry point: kernel(**inputs) -> np.ndarray [200, 2] float32.
"""

import os
import numpy as np

# -------------------- problem constants --------------------
N, F, H, C, HC, ED, E, G = 20000, 128, 8, 64, 512, 4, 320000, 200
NEG = 0.2
NCORES = 8
NS = N // NCORES            # 2500 destination nodes per core
DT = 128                    # destinations per tile (PSUM partition dim)
NT = (NS + DT - 1) // DT    # 20 dst tiles per core
TW = 576                    # table row width (512 feat | 8 a_s | 8 a_d | 48 pad)
NPAD = ((N + 127) // 128) * 128   # 20096 table rows (pad rows zero)
NCH_A = NPAD // 128         # 157 node chunks in phase A
BS = 8                      # max 128-edge chunks per gather batch
# (dma_gather with num_idxs=2048 crashes the exec unit; 1024 is solid)

LAST_EXEC_TIMES = []        # ns per launch (for the test harness)
_PROGRAMS = {}


def _wrap16(a, pad_len=None):
    """dma_gather index layout: idx j sits at [j % 16, j // 16] (int16),
    replicated across the 8 groups of 16 partitions."""
    a = np.asarray(a, np.int64)
    if pad_len is not None and a.size < pad_len:
        a = np.concatenate([a, np.zeros(pad_len - a.size, np.int64)])
    assert a.size % 16 == 0
    w = a.astype(np.int16).reshape(a.size // 16, 16).T
    return np.ascontiguousarray(np.tile(w, (8, 1)))


# ==================== host-side preprocessing ====================

def _prep_params(W, att_s, att_d, att_e, We):
    W = np.asarray(W, np.float32)
    att_s = np.asarray(att_s, np.float32)
    att_d = np.asarray(att_d, np.float32)
    att_e = np.asarray(att_e, np.float32)
    We = np.asarray(We, np.float32)
    As = np.zeros((HC, H), np.float32)
    Ad = np.zeros((HC, H), np.float32)
    for h in range(H):
        As[h * C:(h + 1) * C, h] = att_s[h]
        Ad[h * C:(h + 1) * C, h] = att_d[h]
    Wext = np.zeros((W.shape[0], TW), np.float32)
    Wext[:, :HC] = W
    Wext[:, HC:HC + H] = W @ As
    Wext[:, HC + H:HC + 2 * H] = W @ Ad
    we = np.zeros((ED, H), np.float32)
    for h in range(H):
        we[:, h] = We[:, h * C:(h + 1) * C] @ att_e[h]
    return Wext, we


def _prep_edges(src, dst, edge_attr):
    counts = np.zeros((NCORES, NT), np.int64)
    per_core = []
    for k in range(NCORES):
        m = (dst >= k * NS) & (dst < (k + 1) * NS)
        s, d, ea = src[m], dst[m], edge_attr[m]
        dloc = d - k * NS
        t = dloc // DT
        order = np.argsort(t, kind="stable")
        s, dloc, ea = s[order], dloc[order], ea[order]
        counts[k] = np.bincount(t[order], minlength=NT)
        per_core.append((s, dloc, ea))
    tchunks = np.maximum(1, (counts.max(axis=0) + DT - 1) // DT).astype(np.int64)
    epad = int(tchunks.sum()) * DT
    nchunk = epad // DT
    cores = []
    for k in range(NCORES):
        s, dloc, ea = per_core[k]
        esrc = np.zeros(epad, np.int64)
        edst = np.zeros(epad, np.int64)
        dlocf = np.full(epad, 999.0, np.float32)
        eat = np.zeros((epad, ED), np.float32)
        off = pos = 0
        for ti in range(NT):
            n = int(counts[k, ti])
            sl = slice(pos, pos + n)
            esrc[off:off + n] = s[sl]
            edst[off:off + n] = dloc[sl] + k * NS
            dlocf[off:off + n] = (dloc[sl] - ti * DT).astype(np.float32)
            eat[off:off + n] = ea[sl]
            off += int(tchunks[ti]) * DT
            pos += n
        cores.append(dict(
            esrc_w=_wrap16(esrc),
            edst_w=_wrap16(edst),
            dstloc=np.ascontiguousarray(dlocf.reshape(nchunk, DT).T),
            eattrT=np.ascontiguousarray(eat.T),
            hxidx_w=_wrap16(np.arange(k * NS, k * NS + NT * DT)),
        ))
    return cores, tchunks, epad


def _prep_pool(batch):
    """Per-core pooling matrix over the core's contiguous graph-id window,
    with 1/count baked in. Host overlap-adds windows afterwards (pure unshard
    glue; disjoint for the reference's uniform batch)."""
    batch = np.asarray(batch, np.int64)
    cnt = np.bincount(batch, minlength=G).astype(np.float32)
    g_lo = [int(batch[k * NS:(k + 1) * NS].min()) for k in range(NCORES)]
    g_hi = [int(batch[k * NS:(k + 1) * NS].max()) for k in range(NCORES)]
    ngk = min(max(max(h - l + 1 for l, h in zip(g_lo, g_hi)), 1), 128)
    pmats = []
    for k in range(NCORES):
        Pm = np.zeros((NT * DT, ngk), np.float32)
        bk = batch[k * NS:(k + 1) * NS]
        gl = np.clip(bk - g_lo[k], 0, ngk - 1)
        Pm[np.arange(NS), gl] = 1.0 / np.maximum(cnt[bk], 1.0)
        pm_dev = Pm.reshape(NT, DT, ngk).transpose(1, 0, 2).reshape(DT, NT * ngk)
        pmats.append(np.ascontiguousarray(pm_dev))
    return pmats, g_lo, ngk


# ==================== Bass program builders ====================

def _common_inputs(nc, mybir, epad, nchunk):
    f32, i16 = mybir.dt.float32, mybir.dt.int16
    t = {}
    t["esrc_w"] = nc.dram_tensor("esrc_w", [128, epad // 16], i16, kind="ExternalInput")
    t["edst_w"] = nc.dram_tensor("edst_w", [128, epad // 16], i16, kind="ExternalInput")
    t["hxidx_w"] = nc.dram_tensor("hxidx_w", [128, NT * DT // 16], i16, kind="ExternalInput")
    t["dstloc"] = nc.dram_tensor("dstloc", [128, nchunk], f32, kind="ExternalInput")
    t["eattrT"] = nc.dram_tensor("eattrT", [4, epad], f32, kind="ExternalInput")
    t["iota8"] = nc.dram_tensor("iota8", [128, BS * 128], f32, kind="ExternalInput")
    t["we"] = nc.dram_tensor("we", [4, H], f32, kind="ExternalInput")
    t["b"] = nc.dram_tensor("b", [HC], f32, kind="ExternalInput")
    return t


def _load_common_sbuf(nc, tc, ctx, mybir, t):
    f32, i16 = mybir.dt.float32, mybir.dt.int16
    cpool = ctx.enter_context(tc.tile_pool(name="const", bufs=1))
    sb = {}
    ew = t["esrc_w"].shape[1]
    sb["esrc"] = cpool.tile([128, ew], i16, tag="esrc", name="esrc_sb")
    nc.sync.dma_start(out=sb["esrc"][:], in_=t["esrc_w"].ap())
    sb["edst"] = cpool.tile([128, ew], i16, tag="edst", name="edst_sb")
    nc.sync.dma_start(out=sb["edst"][:], in_=t["edst_w"].ap())
    sb["hxidx"] = cpool.tile([128, NT * DT // 16], i16, tag="hxidx", name="hxidx_sb")
    nc.sync.dma_start(out=sb["hxidx"][:], in_=t["hxidx_w"].ap())
    sb["iota"] = cpool.tile([128, BS, 128], f32, tag="iota", name="iota_sb")
    nc.sync.dma_start(out=sb["iota"][:],
                      in_=t["iota8"].ap().rearrange("p (c x) -> p c x", c=BS))
    sb["web"] = cpool.tile([4, H], f32, tag="web", name="web_sb")
    nc.sync.dma_start(out=sb["web"][:], in_=t["we"].ap())
    sb["b"] = cpool.tile([128, HC], f32, tag="bb", name="b_sb")
    nc.sync.dma_start(out=sb["b"][:], in_=t["b"].ap()[None, :].to_broadcast([128, HC]))
    return sb, cpool


def _edge_phase(nc, mybir, ctx, tc, cfg, tbl, sb, relu, pool_cfg=None):
    """Per-layer edge phase + per-tile epilogue. Stores normalized tiles to
    cfg['store'] (if set) and/or accumulates graph pooling (pool_cfg)."""
    tchunks = cfg["tchunks"]
    f32 = mybir.dt.float32

    gpool = ctx.enter_context(tc.tile_pool(name="gpool", bufs=2))
    spool = ctx.enter_context(tc.tile_pool(name="spool", bufs=2))
    ps_feat_p = ctx.enter_context(tc.tile_pool(name="psf", bufs=2, space="PSUM"))
    ps_stat_p = ctx.enter_context(tc.tile_pool(name="pss", bufs=2, space="PSUM"))
    ps_ae_p = ctx.enter_context(tc.tile_pool(name="psa", bufs=2, space="PSUM"))
    if pool_cfg is not None:
        ps_pool_p = ctx.enter_context(tc.tile_pool(name="psp", bufs=1, space="PSUM"))
        pool_ps = ps_pool_p.tile([pool_cfg["ngk"], 2], f32, tag="poolps", name="pool_ps")

    tbl_ap = tbl.ap()
    tbl_ad = tbl_ap[:, HC:TW]   # cols 512:576 (a_s | a_d | pad) for dst-gather

    chunk0 = 0
    for ti in range(NT):
        nch = int(tchunks[ti])
        ps_feat = ps_feat_p.tile([DT, HC], f32, tag="feat")
        ps_stat = ps_stat_p.tile([DT, 17], f32, tag="stat")
        first_chunk = True
        done = 0
        while done < nch:
            bs = min(BS, nch - done)
            c0 = chunk0 + done
            nidx = bs * DT
            hg = gpool.tile([DT, bs, TW], f32, tag="hg")
            ad = gpool.tile([DT, bs, 64], f32, tag="ad")
            nc.gpsimd.dma_gather(hg[:], tbl_ap,
                                 sb["esrc"][:, c0 * 8: c0 * 8 + nidx // 16],
                                 nidx, nidx, TW)
            nc.gpsimd.dma_gather(ad[:], tbl_ad,
                                 sb["edst"][:, c0 * 8: c0 * 8 + nidx // 16],
                                 nidx, nidx, 64, elem_step=TW)
            eat = spool.tile([4, bs * DT], f32, tag="eat")
            nc.sync.dma_start(out=eat[:],
                              in_=cfg["eattrT"].ap()[:, c0 * DT:(c0 + bs) * DT])
            dl = spool.tile([DT, bs], f32, tag="dl")
            nc.sync.dma_start(out=dl[:], in_=cfg["dstloc"].ap()[:, c0:c0 + bs])

            ps_ae = ps_ae_p.tile([DT, bs * H], f32, tag="ae")
            for c in range(bs):
                nc.tensor.matmul(out=ps_ae[:, c * H:(c + 1) * H],
                                 lhsT=eat[:, c * DT:(c + 1) * DT],
                                 rhs=sb["web"][:], start=True, stop=True)
            ae3 = ps_ae[:].rearrange("p (c h) -> p c h", h=H)
            s = spool.tile([DT, bs, H], f32, tag="s")
            nc.vector.tensor_tensor(out=s[:], in0=hg[:, :, HC:HC + H],
                                    in1=ad[:, :, H:2 * H], op=mybir.AluOpType.add)
            nc.vector.tensor_tensor(out=s[:], in0=s[:], in1=ae3,
                                    op=mybir.AluOpType.add)
            s2 = spool.tile([DT, bs, H], f32, tag="s2")
            nc.vector.tensor_scalar_mul(s2[:], s[:], NEG)
            nc.vector.tensor_tensor(out=s[:], in0=s[:], in1=s2[:],
                                    op=mybir.AluOpType.max)
            wae = spool.tile([DT, bs, 17], f32, tag="wae")
            nc.scalar.activation(wae[:, :, 0:H], s[:],
                                 mybir.ActivationFunctionType.Exp)
            nc.vector.tensor_copy(out=wae[:, :, H:2 * H], in_=ae3)
            nc.vector.memset(wae[:, :, 2 * H:2 * H + 1], 1.0)
            cm = spool.tile([DT, bs, DT], f32, tag="cm")
            nc.vector.tensor_tensor(
                out=cm[:], in0=dl[:, :, None].to_broadcast([DT, bs, DT]),
                in1=sb["iota"][:, 0:bs, :], op=mybir.AluOpType.is_equal)
            hg4 = hg[:, :, 0:HC].rearrange("p c (h x) -> p c h x", h=H)
            nc.vector.tensor_tensor(
                out=hg4, in0=hg4,
                in1=wae[:, :, 0:H].to_broadcast([DT, bs, H, C]),
                op=mybir.AluOpType.mult)
            for c in range(bs):
                last = (done + c == nch - 1)
                nc.tensor.matmul(out=ps_feat[:], lhsT=cm[:, c, :],
                                 rhs=hg[:, c, 0:HC],
                                 start=first_chunk, stop=last,
                                 skip_group_check=True)
                nc.tensor.matmul(out=ps_stat[:], lhsT=cm[:, c, :],
                                 rhs=wae[:, c, :],
                                 start=first_chunk, stop=last,
                                 skip_group_check=True)
                first_chunk = False
            done += bs
        chunk0 += nch

        # ---- per-tile epilogue ----
        hx = spool.tile([DT, 1, TW], f32, tag="hx")
        nc.gpsimd.dma_gather(hx[:], tbl_ap,
                             sb["hxidx"][:, ti * 8:(ti + 1) * 8],
                             DT, DT, TW)
        hx2 = hx[:, 0, :]
        dmax = spool.tile([DT, 1], f32, tag="dmax")
        nc.vector.tensor_scalar_max(dmax[:], ps_stat[:, 16:17], 1.0)
        rdeg = spool.tile([DT, 1], f32, tag="rdeg")
        nc.vector.reciprocal(rdeg[:], dmax[:])
        sl_ = spool.tile([DT, H], f32, tag="sl")
        nc.vector.tensor_tensor(out=sl_[:], in0=ps_stat[:, 8:16],
                                in1=rdeg[:].to_broadcast([DT, H]),
                                op=mybir.AluOpType.mult)
        nc.vector.tensor_tensor(out=sl_[:], in0=sl_[:], in1=hx2[HC:HC + H] if False else hx[:, 0, HC:HC + H],
                                op=mybir.AluOpType.add)
        nc.vector.tensor_tensor(out=sl_[:], in0=sl_[:], in1=hx[:, 0, HC + H:HC + 2 * H],
                                op=mybir.AluOpType.add)
        sl2 = spool.tile([DT, H], f32, tag="sl2")
        nc.vector.tensor_scalar_mul(sl2[:], sl_[:], NEG)
        nc.vector.tensor_tensor(out=sl_[:], in0=sl_[:], in1=sl2[:],
                                op=mybir.AluOpType.max)
        wl = spool.tile([DT, H], f32, tag="wl")
        nc.scalar.activation(wl[:], sl_[:], mybir.ActivationFunctionType.Exp)
        den = spool.tile([DT, H], f32, tag="den")
        nc.vector.tensor_tensor(out=den[:], in0=ps_stat[:, 0:8], in1=wl[:],
                                op=mybir.AluOpType.add)
        rden = spool.tile([DT, H], f32, tag="rden")
        nc.vector.reciprocal(rden[:], den[:])
        out_t = spool.tile([DT, HC], f32, tag="outt")
        o4 = out_t[:].rearrange("p (h x) -> p h x", h=H)
        nc.vector.tensor_tensor(
            out=o4, in0=hx[:, 0, 0:HC].rearrange("p (h x) -> p h x", h=H),
            in1=wl[:, :, None].to_broadcast([DT, H, C]), op=mybir.AluOpType.mult)
        nc.vector.tensor_tensor(out=out_t[:], in0=out_t[:], in1=ps_feat[:],
                                op=mybir.AluOpType.add)
        nc.vector.tensor_tensor(
            out=o4, in0=o4, in1=rden[:, :, None].to_broadcast([DT, H, C]),
            op=mybir.AluOpType.mult)
        nc.vector.tensor_tensor(out=out_t[:], in0=out_t[:], in1=sb["b"][:],
                                op=mybir.AluOpType.add)
        if relu:
            nc.vector.tensor_scalar_max(out_t[:], out_t[:], 0.0)
        if cfg.get("store") is not None:
            nc.sync.dma_start(out=cfg["store"].ap()[ti * DT:(ti + 1) * DT, :],
                              in_=out_t[:])
        if pool_cfg is not None:
            ngk = pool_cfg["ngk"]
            ytile = spool.tile([DT, 2], f32, tag="yt")
            tmp = spool.tile([DT, HC], f32, tag="ytmp")
            for j in range(2):
                nc.vector.tensor_tensor(out=tmp[:], in0=out_t[:],
                                        in1=sb["linw%d" % j][:],
                                        op=mybir.AluOpType.mult)
                nc.vector.tensor_reduce(out=ytile[:, j:j + 1], in_=tmp[:],
                                        axis=mybir.AxisListType.X,
                                        op=mybir.AluOpType.add)
            nc.tensor.matmul(out=pool_ps[:],
                             lhsT=sb["pmat"][:, ti * ngk:(ti + 1) * ngk],
                             rhs=ytile[:], start=(ti == 0), stop=(ti == NT - 1),
                             skip_group_check=True)

    if pool_cfg is not None:
        ysb = spool.tile([pool_cfg["ngk"], 2], f32, tag="ysb")
        nc.vector.tensor_tensor(out=ysb[:], in0=pool_ps[:], in1=sb["linb"][:],
                                op=mybir.AluOpType.add)
        nc.sync.dma_start(out=pool_cfg["yout"].ap()[:, :], in_=ysb[:])


def _build_launch1(meta):
    import concourse.bacc as bacc
    import concourse.tile as tile
    from concourse import mybir
    from contextlib import ExitStack
    from concourse.masks import make_identity

    f32 = mybir.dt.float32
    epad, nchunk, tchunks = meta["epad"], meta["nchunk"], meta["tchunks"]

    nc = bacc.Bacc("TRN2", target_bir_lowering=False, debug=False)
    t = _common_inputs(nc, mybir, epad, nchunk)
    t["xtc"] = nc.dram_tensor("xtc", [NCH_A, 128, 128], f32, kind="ExternalInput")
    t["W1ext"] = nc.dram_tensor("W1ext", [128, TW], f32, kind="ExternalInput")
    t["W2ext"] = nc.dram_tensor("W2ext", [512, TW], f32, kind="ExternalInput")
    tbl = nc.dram_tensor("h1ext", [NPAD, TW], f32)
    x1out = nc.dram_tensor("x1out", [NT * DT, HC], f32, kind="ExternalOutput")
    h2slice = nc.dram_tensor("h2slice", [NT * DT, TW], f32, kind="ExternalOutput")
    parts = os.environ.get("KGAT_L1_PARTS", "AEC")

    with tile.TileContext(nc) as tc, ExitStack() as ctx:
        sb, cpool = _load_common_sbuf(nc, tc, ctx, mybir, t)
        w1sb = cpool.tile([128, TW], f32, tag="w1sb")
        nc.sync.dma_start(out=w1sb[:], in_=t["W1ext"].ap())
        w2sb = []
        for kc in range(4):
            w = cpool.tile([128, TW], f32, tag="w2sb%d" % kc, name="w2sb%d" % kc)
            nc.sync.dma_start(out=w[:], in_=t["W2ext"].ap()[kc * 128:(kc + 1) * 128, :])
            w2sb.append(w)
        ident = cpool.tile([128, 128], f32, tag="ident")
        make_identity(nc, ident[:])

        # ---------------- phase A: h1ext = xpad @ W1ext ----------------
        with tc.tile_pool(name="pA", bufs=3) as pA, \
             tc.tile_pool(name="psA", bufs=2, space="PSUM") as psA_p, \
             tc.tile_pool(name="psA2", bufs=2, space="PSUM") as psA2_p:
            AB = 8
            for t0 in range(0, NCH_A, AB):
                nb = min(AB, NCH_A - t0)
                xb = pA.tile([128, nb, 128], f32, tag="xb")
                nc.sync.dma_start(
                    out=xb[:],
                    in_=t["xtc"].ap()[t0:t0 + nb].rearrange("c f n -> f c n"))
                hb = pA.tile([128, nb, TW], f32, tag="hb")
                for c in range(nb):
                    psA = psA_p.tile([128, HC], f32, tag="psA")
                    psA2 = psA2_p.tile([128, 64], f32, tag="psA2")
                    nc.tensor.matmul(out=psA[:], lhsT=xb[:, c, :],
                                     rhs=w1sb[:, 0:HC], start=True, stop=True)
                    nc.tensor.matmul(out=psA2[:], lhsT=xb[:, c, :],
                                     rhs=w1sb[:, HC:TW], start=True, stop=True)
                    nc.vector.tensor_copy(out=hb[:, c, 0:HC], in_=psA[:])
                    nc.vector.tensor_copy(out=hb[:, c, HC:TW], in_=psA2[:])
                nc.sync.dma_start(
                    out=tbl.ap()[t0 * 128:(t0 + nb) * 128, :].rearrange(
                        "(c p) w -> p c w", p=128),
                    in_=hb[:])
        tc.strict_bb_all_engine_barrier()

        # ---------------- layer-1 edge phase ----------------
        if "E" in parts:
            cfg = dict(tchunks=tchunks, eattrT=t["eattrT"], dstloc=t["dstloc"],
                       store=x1out)
            with ExitStack() as ectx:
                _edge_phase(nc, mybir, ectx, tc, cfg, tbl, sb, relu=True)
            tc.strict_bb_all_engine_barrier()

        # ---------------- phase C: h2slice = x1out @ W2ext ----------------
        if "C" in parts:
          with tc.tile_pool(name="pC", bufs=2) as pC, \
             tc.tile_pool(name="psTr", bufs=2, space="PSUM") as psTr_p, \
             tc.tile_pool(name="psH", bufs=2, space="PSUM") as psH_p, \
             tc.tile_pool(name="psH2", bufs=2, space="PSUM") as psH2_p:
            for ti in range(NT):
                x2 = pC.tile([128, HC], f32, tag="x2")
                nc.sync.dma_start(out=x2[:], in_=x1out.ap()[ti * DT:(ti + 1) * DT, :])
                x2t = pC.tile([128, HC], f32, tag="x2t")
                for kc in range(4):
                    ptr = psTr_p.tile([128, 128], f32, tag="ptr")
                    nc.tensor.transpose(out=ptr[:],
                                        in_=x2[:, kc * 128:(kc + 1) * 128],
                                        identity=ident[:])
                    nc.vector.tensor_copy(out=x2t[:, kc * 128:(kc + 1) * 128],
                                          in_=ptr[:])
                psH = psH_p.tile([128, HC], f32, tag="psH")
                psH2 = psH2_p.tile([128, 64], f32, tag="psH2")
                for kc in range(4):
                    nc.tensor.matmul(out=psH[:],
                                     lhsT=x2t[:, kc * 128:(kc + 1) * 128],
                                     rhs=w2sb[kc][:, 0:HC],
                                     start=(kc == 0), stop=(kc == 3))
                    nc.tensor.matmul(out=psH2[:],
                                     lhsT=x2t[:, kc * 128:(kc + 1) * 128],
                                     rhs=w2sb[kc][:, HC:TW],
                                     start=(kc == 0), stop=(kc == 3))
                h2t = pC.tile([128, TW], f32, tag="h2t")
                nc.vector.tensor_copy(out=h2t[:, 0:HC], in_=psH[:])
                nc.vector.tensor_copy(out=h2t[:, HC:TW], in_=psH2[:])
                nc.sync.dma_start(out=h2slice.ap()[ti * DT:(ti + 1) * DT, :],
                                  in_=h2t[:])

    nc.compile()
    return nc


def _build_launch2(meta):
    import concourse.bacc as bacc
    import concourse.tile as tile
    from concourse import mybir
    from contextlib import ExitStack

    f32 = mybir.dt.float32
    epad, nchunk, tchunks = meta["epad"], meta["nchunk"], meta["tchunks"]
    ngk = meta["ngk"]

    nc = bacc.Bacc("TRN2", target_bir_lowering=False, debug=False)
    t = _common_inputs(nc, mybir, epad, nchunk)
    t["h2ext"] = nc.dram_tensor("h2ext", [NPAD, TW], f32, kind="ExternalInput")
    t["pmat"] = nc.dram_tensor("pmat", [128, NT * ngk], f32, kind="ExternalInput")
    t["lin_w"] = nc.dram_tensor("lin_w", [HC, 2], f32, kind="ExternalInput")
    t["lin_b"] = nc.dram_tensor("lin_b", [2], f32, kind="ExternalInput")
    yout = nc.dram_tensor("yout", [ngk, 2], f32, kind="ExternalOutput")

    with tile.TileContext(nc) as tc, ExitStack() as ctx:
        sb, cpool = _load_common_sbuf(nc, tc, ctx, mybir, t)
        sb["pmat"] = cpool.tile([128, NT * ngk], f32, tag="pm", name="pm_sb")
        nc.sync.dma_start(out=sb["pmat"][:], in_=t["pmat"].ap())
        for j in range(2):
            lw = cpool.tile([128, HC], f32, tag="linw%d" % j, name="linw%d_sb" % j)
            nc.sync.dma_start(
                out=lw[:],
                in_=t["lin_w"].ap()[:, j:j + 1].rearrange("a b -> b a").to_broadcast([128, HC]))
            sb["linw%d" % j] = lw
        sb["linb"] = cpool.tile([ngk, 2], f32, tag="linb", name="linb_sb")
        nc.sync.dma_start(out=sb["linb"][:],
                          in_=t["lin_b"].ap()[None, :].to_broadcast([ngk, 2]))

        cfg = dict(tchunks=tchunks, eattrT=t["eattrT"], dstloc=t["dstloc"],
                   store=None)
        pool_cfg = dict(ngk=ngk, yout=yout)
        with ExitStack() as ectx:
            _edge_phase(nc, mybir, ectx, tc, cfg, t["h2ext"], sb, relu=False,
                        pool_cfg=pool_cfg)

    nc.compile()
    return nc


# ==================== driver ====================

def _install_trace_shim():
    """Dev-only (KGAT_TRACE=1): register the axon NTFF profile hook that this
    image's antenv lacks, and keep profile artifacts local."""
    import sys, types
    try:
        from antenv import axon_hooks  # noqa: F401
        return
    except ImportError:
        pass
    try:
        from trn_agent_boot.trn_boot import _ntff_profile_via_ctypes
        mod = types.ModuleType("antenv.axon_hooks")
        mod._h = _ntff_profile_via_ctypes("/opt/axon/libaxon_pjrt.so")
        mod.set_axon_ntff_profile_hook = lambda h: setattr(mod, "_h", h)
        mod.get_axon_ntff_profile_hook = lambda: mod._h
        sys.modules["antenv.axon_hooks"] = mod
        import antenv
        antenv.axon_hooks = mod
        import concourse.bass_utils as bu
        bu.upload_artifacts = lambda d: d
    except Exception as e:  # pragma: no cover
        print(f"trace shim failed ({e}); falling back to untraced run")


def _run(nc, in_maps, sim_cores=None):
    global LAST_EXEC_TIMES
    if sim_cores is not None:
        from concourse.bass_interp import CoreSim
        out_names = [a.memorylocations[0].name
                     for a in nc.m.functions[0].allocations
                     if getattr(a, "kind", None) == "ExternalOutput"]
        outs = [None] * len(in_maps)
        for ci in sim_cores:
            s = CoreSim(nc, trace=False)
            for k, v in in_maps[ci].items():
                s.tensor(k)[:] = v
            s.simulate(check_with_hw=False)
            outs[ci] = {n: np.array(s.tensor(n)) for n in out_names}
        return outs
    trace = os.environ.get("KGAT_TRACE", "0") == "1"
    if trace:
        _install_trace_shim()
    from concourse.bass_utils import run_bass_kernel_spmd
    import time as _time
    t0 = _time.time()
    res = run_bass_kernel_spmd(nc, in_maps, list(range(NCORES)), trace=trace)
    if os.environ.get("KGAT_VERBOSE", "0") == "1":
        print(f"launch wall {_time.time() - t0:.2f}s exec_ns={res.exec_time_ns}")
    if res.exec_time_ns is not None:
        LAST_EXEC_TIMES.append(res.exec_time_ns)
    return res.results


def _get_program(which, meta):
    key = (which, meta["epad"], tuple(meta["tchunks"]), meta.get("ngk"))
    if key not in _PROGRAMS:
        _PROGRAMS[key] = (_build_launch1 if which == 1 else _build_launch2)(meta)
    return _PROGRAMS[key]


def kernel(**inputs):
    global LAST_EXEC_TIMES
    LAST_EXEC_TIMES = []
    sim = os.environ.get("KGAT_SIM", "0") == "1"
    sim_cores = list(range(NCORES)) if sim else None

    x = np.asarray(inputs["x"], np.float32)
    ei = np.asarray(inputs["edge_index"], np.int64)
    batch = np.asarray(inputs["batch"], np.int64)
    edge_attr = np.asarray(inputs["edge_attr"], np.float32)

    W1ext, we1 = _prep_params(inputs["W1"], inputs["att_src1"], inputs["att_dst1"],
                              inputs["att_edge1"], inputs["We1"])
    W2ext, we2 = _prep_params(inputs["W2"], inputs["att_src2"], inputs["att_dst2"],
                              inputs["att_edge2"], inputs["We2"])
    b1 = np.asarray(inputs["b1"], np.float32)
    b2 = np.asarray(inputs["b2"], np.float32)
    lin_w = np.asarray(inputs["lin_w"], np.float32)
    lin_b = np.asarray(inputs["lin_b"], np.float32)

    cores, tchunks, epad = _prep_edges(ei[0], ei[1], edge_attr)
    pmats, g_lo, ngk = _prep_pool(batch)
    nchunk = epad // DT

    xpad = np.zeros((NPAD, F), np.float32)
    xpad[:N] = x
    xtc = np.ascontiguousarray(xpad.reshape(NCH_A, 128, F).transpose(0, 2, 1))
    iota8 = np.ascontiguousarray(
        np.tile(np.arange(128, dtype=np.float32), (128, BS)))

    meta = dict(epad=epad, nchunk=nchunk, tchunks=tchunks, ngk=ngk)

    def common_maps(k):
        ck = cores[k]
        return dict(esrc_w=ck["esrc_w"], edst_w=ck["edst_w"],
                    hxidx_w=ck["hxidx_w"], dstloc=ck["dstloc"],
                    eattrT=ck["eattrT"], iota8=iota8)

    nc1 = _get_program(1, meta)
    in_maps1 = []
    for k in range(NCORES):
        m = common_maps(k)
        m.update(xtc=xtc, W1ext=W1ext, W2ext=W2ext, we=we1, b=b1)
        in_maps1.append(m)
    res1 = _run(nc1, in_maps1, sim_cores=sim_cores)

    h2full = np.concatenate([res1[k]["h2slice"][:NS] for k in range(NCORES)], 0)
    h2pad = np.zeros((NPAD, TW), np.float32)
    h2pad[:N] = h2full

    nc2 = _get_program(2, meta)
    in_maps2 = []
    for k in range(NCORES):
        m = common_maps(k)
        m.update(h2ext=h2pad, pmat=pmats[k], we=we2, b=b2,
                 lin_w=lin_w, lin_b=lin_b)
        in_maps2.append(m)
    res2 = _run(nc2, in_maps2, sim_cores=sim_cores)

    y = np.zeros((G, 2), np.float32)
    for k in range(NCORES):
        yk = res2[k]["yout"]
        for gi in range(ngk):
            g = g_lo[k] + gi
            if g < G:
                y[g] += yk[gi]
    return (y + lin_b[None, :]).astype(np.float32)


# revision 12
# speedup vs baseline: 1.0651x; 1.0651x over previous
"""Trainium2 Bass kernel for a 2-layer GAT (PyG GATConv, concat heads) +
global mean pool + linear head, distributed over 8 NeuronCores.

Strategy (self-contained; shapes hardcoded for this problem):
  - Destination-shard nodes across the 8 cores (2500 each); a core owns the
    edges whose destination lies in its slice. Graph pooling is shard-local.
  - he = edge_attr @ We is never materialized: its only use is
    a_e = he . att_e, which collapses to a_e = edge_attr @ we with the [4, 8]
    matrix we[d, h] = We[d, h*64:(h+1)*64] @ att_e[h] (host weight folding).
  - Self-loops (fill_value='mean') are folded analytically: the self-loop's
    a_e equals segment_sum(a_e)/max(deg, 1); both stats are accumulated as
    extra columns of the edge-aggregation matmul.
  - Softmax without max-subtraction: out_i = sum_e exp(r_e) h_src / sum exp(r_e)
    is mathematically identical to the max-normalized form (logits are O(10)).
  - Edge aggregation per 128-destination tile = PE matmul with the one-hot
    scatter matrix C[e, d] = (dst_local[e] == d), built on DVE from an iota
    constant; messages are dma_gather'ed source rows scaled by exp-logits.
  - Two SPMD launches: launch 1 builds the layer-1 node table x @ W1ext
    (features + a_s + a_d columns), runs the layer-1 edge phase, and emits
    each core's slice of the layer-2 table h2 = relu(out1) @ W2ext. The host
    concatenates slices (pure data movement) and launch 2 runs the layer-2
    edge phase + pooling + linear head.

Graded entry point: kernel(**inputs) -> np.ndarray [200, 2] float32.
"""

import os
import numpy as np

# -------------------- problem constants --------------------
N, F, H, C, HC, ED, E, G = 20000, 128, 8, 64, 512, 4, 320000, 200
NEG = 0.2
NCORES = 8
NS = N // NCORES            # 2500 destination nodes per core
DT = 128                    # destinations per tile (PSUM partition dim)
NT = (NS + DT - 1) // DT    # 20 dst tiles per core
TW = 576                    # table row width (512 feat | 8 a_s | 8 a_d | 48 pad)
NPAD = ((N + 127) // 128) * 128   # 20096 table rows (pad rows zero)
NCH_A = NPAD // 128         # 157 node chunks in phase A
BS = 8                      # max 128-edge chunks per gather batch
# (dma_gather with num_idxs=2048 crashes the exec unit; 1024 is solid)

LAST_EXEC_TIMES = []        # ns per launch (for the test harness)
_PROGRAMS = {}


def _wrap16(a, pad_len=None):
    """dma_gather index layout: idx j sits at [j % 16, j // 16] (int16),
    replicated across the 8 groups of 16 partitions."""
    a = np.asarray(a, np.int64)
    if pad_len is not None and a.size < pad_len:
        a = np.concatenate([a, np.zeros(pad_len - a.size, np.int64)])
    assert a.size % 16 == 0
    w = a.astype(np.int16).reshape(a.size // 16, 16).T
    return np.ascontiguousarray(np.tile(w, (8, 1)))


# ==================== host-side preprocessing ====================

def _prep_params(W, att_s, att_d, att_e, We):
    W = np.asarray(W, np.float32)
    att_s = np.asarray(att_s, np.float32)
    att_d = np.asarray(att_d, np.float32)
    att_e = np.asarray(att_e, np.float32)
    We = np.asarray(We, np.float32)
    As = np.zeros((HC, H), np.float32)
    Ad = np.zeros((HC, H), np.float32)
    for h in range(H):
        As[h * C:(h + 1) * C, h] = att_s[h]
        Ad[h * C:(h + 1) * C, h] = att_d[h]
    Wext = np.zeros((W.shape[0], TW), np.float32)
    Wext[:, :HC] = W
    Wext[:, HC:HC + H] = W @ As
    Wext[:, HC + H:HC + 2 * H] = W @ Ad
    we = np.zeros((ED, H), np.float32)
    for h in range(H):
        we[:, h] = We[:, h * C:(h + 1) * C] @ att_e[h]
    return Wext, we


def _prep_edges(src, dst, edge_attr):
    counts = np.zeros((NCORES, NT), np.int64)
    per_core = []
    for k in range(NCORES):
        m = (dst >= k * NS) & (dst < (k + 1) * NS)
        s, d, ea = src[m], dst[m], edge_attr[m]
        dloc = d - k * NS
        t = dloc // DT
        order = np.argsort(t, kind="stable")
        s, dloc, ea = s[order], dloc[order], ea[order]
        counts[k] = np.bincount(t[order], minlength=NT)
        per_core.append((s, dloc, ea))
    tchunks = np.maximum(1, (counts.max(axis=0) + DT - 1) // DT).astype(np.int64)
    epad = int(tchunks.sum()) * DT
    nchunk = epad // DT
    cores = []
    for k in range(NCORES):
        s, dloc, ea = per_core[k]
        esrc = np.zeros(epad, np.int64)
        edst = np.zeros(epad, np.int64)
        dlocf = np.full(epad, 999.0, np.float32)
        eat = np.zeros((epad, ED), np.float32)
        off = pos = 0
        for ti in range(NT):
            n = int(counts[k, ti])
            sl = slice(pos, pos + n)
            esrc[off:off + n] = s[sl]
            edst[off:off + n] = dloc[sl] + k * NS
            dlocf[off:off + n] = (dloc[sl] - ti * DT).astype(np.float32)
            eat[off:off + n] = ea[sl]
            off += int(tchunks[ti]) * DT
            pos += n
        cores.append(dict(
            esrc_w=_wrap16(esrc),
            dstloc=np.ascontiguousarray(dlocf.reshape(nchunk, DT).T),
            dstlocT=np.ascontiguousarray(dlocf.reshape(nchunk, DT)),
            eattrT=np.ascontiguousarray(eat.T),
            hxidx_w=_wrap16(np.arange(k * NS, k * NS + NT * DT)),
        ))
    return cores, tchunks, epad


def _prep_pool(batch):
    """Per-core pooling matrix over the core's contiguous graph-id window,
    with 1/count baked in. Host overlap-adds windows afterwards (pure unshard
    glue; disjoint for the reference's uniform batch)."""
    batch = np.asarray(batch, np.int64)
    cnt = np.bincount(batch, minlength=G).astype(np.float32)
    g_lo = [int(batch[k * NS:(k + 1) * NS].min()) for k in range(NCORES)]
    g_hi = [int(batch[k * NS:(k + 1) * NS].max()) for k in range(NCORES)]
    ngk = min(max(max(h - l + 1 for l, h in zip(g_lo, g_hi)), 1), 128)
    pmats = []
    for k in range(NCORES):
        Pm = np.zeros((NT * DT, ngk), np.float32)
        bk = batch[k * NS:(k + 1) * NS]
        gl = np.clip(bk - g_lo[k], 0, ngk - 1)
        Pm[np.arange(NS), gl] = 1.0 / np.maximum(cnt[bk], 1.0)
        pm_dev = Pm.reshape(NT, DT, ngk).transpose(1, 0, 2).reshape(DT, NT * ngk)
        pmats.append(np.ascontiguousarray(pm_dev))
    return pmats, g_lo, ngk


# ==================== Bass program builders ====================

def _common_inputs(nc, mybir, epad, nchunk):
    f32, i16 = mybir.dt.float32, mybir.dt.int16
    t = {}
    t["esrc_w"] = nc.dram_tensor("esrc_w", [128, epad // 16], i16, kind="ExternalInput")
    t["hxidx_w"] = nc.dram_tensor("hxidx_w", [128, NT * DT // 16], i16, kind="ExternalInput")
    t["dstloc"] = nc.dram_tensor("dstloc", [128, nchunk], f32, kind="ExternalInput")
    t["dstlocT"] = nc.dram_tensor("dstlocT", [nchunk, 128], f32, kind="ExternalInput")
    t["iotap"] = nc.dram_tensor("iotap", [128, 1], f32, kind="ExternalInput")
    t["eattrT"] = nc.dram_tensor("eattrT", [4, epad], f32, kind="ExternalInput")
    t["iota8"] = nc.dram_tensor("iota8", [128, BS * 128], f32, kind="ExternalInput")
    t["we"] = nc.dram_tensor("we", [4, H], f32, kind="ExternalInput")
    t["b"] = nc.dram_tensor("b", [HC], f32, kind="ExternalInput")
    return t


def _load_common_sbuf(nc, tc, ctx, mybir, t):
    f32, i16 = mybir.dt.float32, mybir.dt.int16
    cpool = ctx.enter_context(tc.tile_pool(name="const", bufs=1))
    sb = {}
    ew = t["esrc_w"].shape[1]
    sb["esrc"] = cpool.tile([128, ew], i16, tag="esrc", name="esrc_sb")
    nc.sync.dma_start(out=sb["esrc"][:], in_=t["esrc_w"].ap())
    sb["hxidx"] = cpool.tile([128, NT * DT // 16], i16, tag="hxidx", name="hxidx_sb")
    nc.sync.dma_start(out=sb["hxidx"][:], in_=t["hxidx_w"].ap())
    sb["iota"] = cpool.tile([128, BS, 128], f32, tag="iota", name="iota_sb")
    nc.sync.dma_start(out=sb["iota"][:],
                      in_=t["iota8"].ap().rearrange("p (c x) -> p c x", c=BS))
    sb["web"] = cpool.tile([4, H], f32, tag="web", name="web_sb")
    nc.sync.dma_start(out=sb["web"][:], in_=t["we"].ap())
    sb["iotap"] = cpool.tile([128, 1], f32, tag="iotap", name="iotap_sb")
    nc.sync.dma_start(out=sb["iotap"][:], in_=t["iotap"].ap())
    sb["b"] = cpool.tile([128, HC], f32, tag="bb", name="b_sb")
    nc.sync.dma_start(out=sb["b"][:], in_=t["b"].ap()[None, :].to_broadcast([128, HC]))
    return sb, cpool


def _edge_phase(nc, mybir, ctx, tc, cfg, tbl, sb, relu, pool_cfg=None):
    """Per-layer edge phase + per-tile epilogue. Stores normalized tiles to
    cfg['store'] (if set) and/or accumulates graph pooling (pool_cfg)."""
    tchunks = cfg["tchunks"]
    f32 = mybir.dt.float32

    gpool = ctx.enter_context(tc.tile_pool(name="gpool", bufs=2))
    spool = ctx.enter_context(tc.tile_pool(name="spool", bufs=2))
    ps_feat_p = ctx.enter_context(tc.tile_pool(name="psf", bufs=2, space="PSUM"))
    ps_stat_p = ctx.enter_context(tc.tile_pool(name="pss", bufs=2, space="PSUM"))
    ps_ae_p = ctx.enter_context(tc.tile_pool(name="psa", bufs=2, space="PSUM"))
    if pool_cfg is not None:
        ps_pool_p = ctx.enter_context(tc.tile_pool(name="psp", bufs=1, space="PSUM"))
        pool_ps = ps_pool_p.tile([pool_cfg["ngk"], 2], f32, tag="poolps", name="pool_ps")

    tbl_ap = tbl.ap()

    chunk0 = 0
    for ti in range(NT):
        nch = int(tchunks[ti])
        ps_feat = ps_feat_p.tile([DT, HC], f32, tag="feat")
        ps_stat = ps_stat_p.tile([DT, 17], f32, tag="stat")
        # local table rows (features + a_s + a_d) for this dst tile; the a_d
        # column block doubles as the rhs of the per-chunk a_d broadcast matmul
        hx = spool.tile([DT, 1, TW], f32, tag="hx")
        nc.gpsimd.dma_gather(hx[:], tbl_ap,
                             sb["hxidx"][:, ti * 8:(ti + 1) * 8],
                             DT, DT, TW)
        first_chunk = True
        done = 0
        while done < nch:
            bs = min(BS, nch - done)
            c0 = chunk0 + done
            nidx = bs * DT
            hg = gpool.tile([DT, bs, TW], f32, tag="hg")
            nc.gpsimd.dma_gather(hg[:], tbl_ap,
                                 sb["esrc"][:, c0 * 8: c0 * 8 + nidx // 16],
                                 nidx, nidx, TW)
            eat = spool.tile([4, bs * DT], f32, tag="eat")
            nc.sync.dma_start(out=eat[:],
                              in_=cfg["eattrT"].ap()[:, c0 * DT:(c0 + bs) * DT])
            dl = spool.tile([DT, bs], f32, tag="dl")
            nc.sync.dma_start(out=dl[:], in_=cfg["dstloc"].ap()[:, c0:c0 + bs])
            # dst-locals replicated across partitions (for the transposed
            # scatter matrix); the partition-broadcast comes from the DMA
            dlb = spool.tile([DT, bs, DT], f32, tag="dlb")
            nc.sync.dma_start(
                out=dlb[:],
                in_=cfg["dstlocT"].ap()[None, c0:c0 + bs, :].to_broadcast(
                    [DT, bs, DT]))
            ct = spool.tile([DT, bs, DT], f32, tag="ct")
            nc.vector.tensor_scalar(ct[:], dlb[:], sb["iotap"][:], None,
                                    mybir.AluOpType.is_equal)

            ps_ae = ps_ae_p.tile([DT, bs, 16], f32, tag="ae")
            for c in range(bs):
                nc.tensor.matmul(out=ps_ae[:, c, 0:H],
                                 lhsT=eat[:, c * DT:(c + 1) * DT],
                                 rhs=sb["web"][:], start=True, stop=True)
                # a_d[dst_e] broadcast: CT.T @ a_d_tile
                nc.tensor.matmul(out=ps_ae[:, c, H:2 * H],
                                 lhsT=ct[:, c, :],
                                 rhs=hx[:, 0, HC + H:HC + 2 * H],
                                 start=True, stop=True)
            ae3 = ps_ae[:, :, 0:H]
            s = spool.tile([DT, bs, H], f32, tag="s")
            nc.vector.tensor_tensor(out=s[:], in0=hg[:, :, HC:HC + H],
                                    in1=ps_ae[:, :, H:2 * H],
                                    op=mybir.AluOpType.add)
            nc.vector.tensor_tensor(out=s[:], in0=s[:], in1=ae3,
                                    op=mybir.AluOpType.add)
            s2 = spool.tile([DT, bs, H], f32, tag="s2")
            nc.vector.tensor_scalar_mul(s2[:], s[:], NEG)
            nc.vector.tensor_tensor(out=s[:], in0=s[:], in1=s2[:],
                                    op=mybir.AluOpType.max)
            wae = spool.tile([DT, bs, 17], f32, tag="wae")
            nc.scalar.activation(wae[:, :, 0:H], s[:],
                                 mybir.ActivationFunctionType.Exp)
            nc.vector.tensor_copy(out=wae[:, :, H:2 * H], in_=ae3)
            nc.vector.memset(wae[:, :, 2 * H:2 * H + 1], 1.0)
            cm = spool.tile([DT, bs, DT], f32, tag="cm")
            nc.vector.tensor_tensor(
                out=cm[:], in0=dl[:, :, None].to_broadcast([DT, bs, DT]),
                in1=sb["iota"][:, 0:bs, :], op=mybir.AluOpType.is_equal)
            hg4 = hg[:, :, 0:HC].rearrange("p c (h x) -> p c h x", h=H)
            nc.vector.tensor_tensor(
                out=hg4, in0=hg4,
                in1=wae[:, :, 0:H].to_broadcast([DT, bs, H, C]),
                op=mybir.AluOpType.mult)
            for c in range(bs):
                last = (done + c == nch - 1)
                nc.tensor.matmul(out=ps_feat[:], lhsT=cm[:, c, :],
                                 rhs=hg[:, c, 0:HC],
                                 start=first_chunk, stop=last,
                                 skip_group_check=True)
                nc.tensor.matmul(out=ps_stat[:], lhsT=cm[:, c, :],
                                 rhs=wae[:, c, :],
                                 start=first_chunk, stop=last,
                                 skip_group_check=True)
                first_chunk = False
            done += bs
        chunk0 += nch

        # ---- per-tile epilogue ----
        dmax = spool.tile([DT, 1], f32, tag="dmax")
        nc.vector.tensor_scalar_max(dmax[:], ps_stat[:, 16:17], 1.0)
        rdeg = spool.tile([DT, 1], f32, tag="rdeg")
        nc.vector.reciprocal(rdeg[:], dmax[:])
        sl_ = spool.tile([DT, H], f32, tag="sl")
        nc.vector.tensor_tensor(out=sl_[:], in0=ps_stat[:, 8:16],
                                in1=rdeg[:].to_broadcast([DT, H]),
                                op=mybir.AluOpType.mult)
        nc.vector.tensor_tensor(out=sl_[:], in0=sl_[:], in1=hx[:, 0, HC:HC + H],
                                op=mybir.AluOpType.add)
        nc.vector.tensor_tensor(out=sl_[:], in0=sl_[:], in1=hx[:, 0, HC + H:HC + 2 * H],
                                op=mybir.AluOpType.add)
        sl2 = spool.tile([DT, H], f32, tag="sl2")
        nc.vector.tensor_scalar_mul(sl2[:], sl_[:], NEG)
        nc.vector.tensor_tensor(out=sl_[:], in0=sl_[:], in1=sl2[:],
                                op=mybir.AluOpType.max)
        wl = spool.tile([DT, H], f32, tag="wl")
        nc.scalar.activation(wl[:], sl_[:], mybir.ActivationFunctionType.Exp)
        den = spool.tile([DT, H], f32, tag="den")
        nc.vector.tensor_tensor(out=den[:], in0=ps_stat[:, 0:8], in1=wl[:],
                                op=mybir.AluOpType.add)
        rden = spool.tile([DT, H], f32, tag="rden")
        nc.vector.reciprocal(rden[:], den[:])
        out_t = spool.tile([DT, HC], f32, tag="outt")
        o4 = out_t[:].rearrange("p (h x) -> p h x", h=H)
        nc.vector.tensor_tensor(
            out=o4, in0=hx[:, 0, 0:HC].rearrange("p (h x) -> p h x", h=H),
            in1=wl[:, :, None].to_broadcast([DT, H, C]), op=mybir.AluOpType.mult)
        nc.vector.tensor_tensor(out=out_t[:], in0=out_t[:], in1=ps_feat[:],
                                op=mybir.AluOpType.add)
        nc.vector.tensor_tensor(
            out=o4, in0=o4, in1=rden[:, :, None].to_broadcast([DT, H, C]),
            op=mybir.AluOpType.mult)
        nc.vector.tensor_tensor(out=out_t[:], in0=out_t[:], in1=sb["b"][:],
                                op=mybir.AluOpType.add)
        if relu:
            nc.vector.tensor_scalar_max(out_t[:], out_t[:], 0.0)
        if cfg.get("store") is not None:
            nc.sync.dma_start(out=cfg["store"].ap()[ti * DT:(ti + 1) * DT, :],
                              in_=out_t[:])
        if pool_cfg is not None:
            ngk = pool_cfg["ngk"]
            ytile = spool.tile([DT, 2], f32, tag="yt")
            tmp = spool.tile([DT, HC], f32, tag="ytmp")
            for j in range(2):
                nc.vector.tensor_tensor(out=tmp[:], in0=out_t[:],
                                        in1=sb["linw%d" % j][:],
                                        op=mybir.AluOpType.mult)
                nc.vector.tensor_reduce(out=ytile[:, j:j + 1], in_=tmp[:],
                                        axis=mybir.AxisListType.X,
                                        op=mybir.AluOpType.add)
            nc.tensor.matmul(out=pool_ps[:],
                             lhsT=sb["pmat"][:, ti * ngk:(ti + 1) * ngk],
                             rhs=ytile[:], start=(ti == 0), stop=(ti == NT - 1),
                             skip_group_check=True)

    if pool_cfg is not None:
        ysb = spool.tile([pool_cfg["ngk"], 2], f32, tag="ysb")
        nc.vector.tensor_tensor(out=ysb[:], in0=pool_ps[:], in1=sb["linb"][:],
                                op=mybir.AluOpType.add)
        nc.sync.dma_start(out=pool_cfg["yout"].ap()[:, :], in_=ysb[:])


def _build_launch1(meta):
    import concourse.bacc as bacc
    import concourse.tile as tile
    from concourse import mybir
    from contextlib import ExitStack
    from concourse.masks import make_identity

    f32 = mybir.dt.float32
    epad, nchunk, tchunks = meta["epad"], meta["nchunk"], meta["tchunks"]

    nc = bacc.Bacc("TRN2", target_bir_lowering=False, debug=False)
    t = _common_inputs(nc, mybir, epad, nchunk)
    t["xtc"] = nc.dram_tensor("xtc", [NCH_A, 128, 128], f32, kind="ExternalInput")
    t["W1ext"] = nc.dram_tensor("W1ext", [128, TW], f32, kind="ExternalInput")
    t["W2ext"] = nc.dram_tensor("W2ext", [512, TW], f32, kind="ExternalInput")
    tbl = nc.dram_tensor("h1ext", [NPAD, TW], f32)
    x1out = nc.dram_tensor("x1out", [NT * DT, HC], f32, kind="ExternalOutput")
    h2slice = nc.dram_tensor("h2slice", [NT * DT, TW], f32, kind="ExternalOutput")
    parts = os.environ.get("KGAT_L1_PARTS", "AEC")

    with tile.TileContext(nc) as tc, ExitStack() as ctx:
        sb, cpool = _load_common_sbuf(nc, tc, ctx, mybir, t)
        w1sb = cpool.tile([128, TW], f32, tag="w1sb")
        nc.sync.dma_start(out=w1sb[:], in_=t["W1ext"].ap())
        w2sb = []
        for kc in range(4):
            w = cpool.tile([128, TW], f32, tag="w2sb%d" % kc, name="w2sb%d" % kc)
            nc.sync.dma_start(out=w[:], in_=t["W2ext"].ap()[kc * 128:(kc + 1) * 128, :])
            w2sb.append(w)
        ident = cpool.tile([128, 128], f32, tag="ident")
        make_identity(nc, ident[:])

        # ---------------- phase A: h1ext = xpad @ W1ext ----------------
        with tc.tile_pool(name="pA", bufs=3) as pA, \
             tc.tile_pool(name="psA", bufs=2, space="PSUM") as psA_p, \
             tc.tile_pool(name="psA2", bufs=2, space="PSUM") as psA2_p:
            AB = 8
            for t0 in range(0, NCH_A, AB):
                nb = min(AB, NCH_A - t0)
                xb = pA.tile([128, nb, 128], f32, tag="xb")
                nc.sync.dma_start(
                    out=xb[:],
                    in_=t["xtc"].ap()[t0:t0 + nb].rearrange("c f n -> f c n"))
                hb = pA.tile([128, nb, TW], f32, tag="hb")
                for c in range(nb):
                    psA = psA_p.tile([128, HC], f32, tag="psA")
                    psA2 = psA2_p.tile([128, 64], f32, tag="psA2")
                    nc.tensor.matmul(out=psA[:], lhsT=xb[:, c, :],
                                     rhs=w1sb[:, 0:HC], start=True, stop=True)
                    nc.tensor.matmul(out=psA2[:], lhsT=xb[:, c, :],
                                     rhs=w1sb[:, HC:TW], start=True, stop=True)
                    nc.vector.tensor_copy(out=hb[:, c, 0:HC], in_=psA[:])
                    nc.vector.tensor_copy(out=hb[:, c, HC:TW], in_=psA2[:])
                nc.sync.dma_start(
                    out=tbl.ap()[t0 * 128:(t0 + nb) * 128, :].rearrange(
                        "(c p) w -> p c w", p=128),
                    in_=hb[:])
        tc.strict_bb_all_engine_barrier()

        # ---------------- layer-1 edge phase ----------------
        if "E" in parts:
            cfg = dict(tchunks=tchunks, eattrT=t["eattrT"], dstloc=t["dstloc"],
                       dstlocT=t["dstlocT"], store=x1out)
            with ExitStack() as ectx:
                _edge_phase(nc, mybir, ectx, tc, cfg, tbl, sb, relu=True)
            tc.strict_bb_all_engine_barrier()

        # ---------------- phase C: h2slice = x1out @ W2ext ----------------
        if "C" in parts:
          with tc.tile_pool(name="pC", bufs=2) as pC, \
             tc.tile_pool(name="psTr", bufs=2, space="PSUM") as psTr_p, \
             tc.tile_pool(name="psH", bufs=2, space="PSUM") as psH_p, \
             tc.tile_pool(name="psH2", bufs=2, space="PSUM") as psH2_p:
            for ti in range(NT):
                x2 = pC.tile([128, HC], f32, tag="x2")
                nc.sync.dma_start(out=x2[:], in_=x1out.ap()[ti * DT:(ti + 1) * DT, :])
                x2t = pC.tile([128, HC], f32, tag="x2t")
                for kc in range(4):
                    ptr = psTr_p.tile([128, 128], f32, tag="ptr")
                    nc.tensor.transpose(out=ptr[:],
                                        in_=x2[:, kc * 128:(kc + 1) * 128],
                                        identity=ident[:])
                    nc.vector.tensor_copy(out=x2t[:, kc * 128:(kc + 1) * 128],
                                          in_=ptr[:])
                psH = psH_p.tile([128, HC], f32, tag="psH")
                psH2 = psH2_p.tile([128, 64], f32, tag="psH2")
                for kc in range(4):
                    nc.tensor.matmul(out=psH[:],
                                     lhsT=x2t[:, kc * 128:(kc + 1) * 128],
                                     rhs=w2sb[kc][:, 0:HC],
                                     start=(kc == 0), stop=(kc == 3))
                    nc.tensor.matmul(out=psH2[:],
                                     lhsT=x2t[:, kc * 128:(kc + 1) * 128],
                                     rhs=w2sb[kc][:, HC:TW],
                                     start=(kc == 0), stop=(kc == 3))
                h2t = pC.tile([128, TW], f32, tag="h2t")
                nc.vector.tensor_copy(out=h2t[:, 0:HC], in_=psH[:])
                nc.vector.tensor_copy(out=h2t[:, HC:TW], in_=psH2[:])
                nc.sync.dma_start(out=h2slice.ap()[ti * DT:(ti + 1) * DT, :],
                                  in_=h2t[:])

    nc.compile()
    return nc


def _build_launch2(meta):
    import concourse.bacc as bacc
    import concourse.tile as tile
    from concourse import mybir
    from contextlib import ExitStack

    f32 = mybir.dt.float32
    epad, nchunk, tchunks = meta["epad"], meta["nchunk"], meta["tchunks"]
    ngk = meta["ngk"]

    nc = bacc.Bacc("TRN2", target_bir_lowering=False, debug=False)
    t = _common_inputs(nc, mybir, epad, nchunk)
    t["h2ext"] = nc.dram_tensor("h2ext", [NPAD, TW], f32, kind="ExternalInput")
    t["pmat"] = nc.dram_tensor("pmat", [128, NT * ngk], f32, kind="ExternalInput")
    t["lin_w"] = nc.dram_tensor("lin_w", [HC, 2], f32, kind="ExternalInput")
    t["lin_b"] = nc.dram_tensor("lin_b", [2], f32, kind="ExternalInput")
    yout = nc.dram_tensor("yout", [ngk, 2], f32, kind="ExternalOutput")

    with tile.TileContext(nc) as tc, ExitStack() as ctx:
        sb, cpool = _load_common_sbuf(nc, tc, ctx, mybir, t)
        sb["pmat"] = cpool.tile([128, NT * ngk], f32, tag="pm", name="pm_sb")
        nc.sync.dma_start(out=sb["pmat"][:], in_=t["pmat"].ap())
        for j in range(2):
            lw = cpool.tile([128, HC], f32, tag="linw%d" % j, name="linw%d_sb" % j)
            nc.sync.dma_start(
                out=lw[:],
                in_=t["lin_w"].ap()[:, j:j + 1].rearrange("a b -> b a").to_broadcast([128, HC]))
            sb["linw%d" % j] = lw
        sb["linb"] = cpool.tile([ngk, 2], f32, tag="linb", name="linb_sb")
        nc.sync.dma_start(out=sb["linb"][:],
                          in_=t["lin_b"].ap()[None, :].to_broadcast([ngk, 2]))

        cfg = dict(tchunks=tchunks, eattrT=t["eattrT"], dstloc=t["dstloc"],
                   dstlocT=t["dstlocT"], store=None)
        pool_cfg = dict(ngk=ngk, yout=yout)
        with ExitStack() as ectx:
            _edge_phase(nc, mybir, ectx, tc, cfg, t["h2ext"], sb, relu=False,
                        pool_cfg=pool_cfg)

    nc.compile()
    return nc


# ==================== driver ====================

def _install_trace_shim():
    """Dev-only (KGAT_TRACE=1): register the axon NTFF profile hook that this
    image's antenv lacks, and keep profile artifacts local."""
    import sys, types
    try:
        from antenv import axon_hooks  # noqa: F401
        return
    except ImportError:
        pass
    try:
        from trn_agent_boot.trn_boot import _ntff_profile_via_ctypes
        mod = types.ModuleType("antenv.axon_hooks")
        mod._h = _ntff_profile_via_ctypes("/opt/axon/libaxon_pjrt.so")
        mod.set_axon_ntff_profile_hook = lambda h: setattr(mod, "_h", h)
        mod.get_axon_ntff_profile_hook = lambda: mod._h
        sys.modules["antenv.axon_hooks"] = mod
        import antenv
        antenv.axon_hooks = mod
        import concourse.bass_utils as bu
        bu.upload_artifacts = lambda d: d
    except Exception as e:  # pragma: no cover
        print(f"trace shim failed ({e}); falling back to untraced run")


def _run(nc, in_maps, sim_cores=None):
    global LAST_EXEC_TIMES
    if sim_cores is not None:
        from concourse.bass_interp import CoreSim
        out_names = [a.memorylocations[0].name
                     for a in nc.m.functions[0].allocations
                     if getattr(a, "kind", None) == "ExternalOutput"]
        outs = [None] * len(in_maps)
        for ci in sim_cores:
            s = CoreSim(nc, trace=False)
            for k, v in in_maps[ci].items():
                s.tensor(k)[:] = v
            s.simulate(check_with_hw=False)
            outs[ci] = {n: np.array(s.tensor(n)) for n in out_names}
        return outs
    trace = os.environ.get("KGAT_TRACE", "0") == "1"
    if trace:
        _install_trace_shim()
    from concourse.bass_utils import run_bass_kernel_spmd
    import time as _time
    t0 = _time.time()
    res = run_bass_kernel_spmd(nc, in_maps, list(range(NCORES)), trace=trace)
    if os.environ.get("KGAT_VERBOSE", "0") == "1":
        print(f"launch wall {_time.time() - t0:.2f}s exec_ns={res.exec_time_ns}")
    if res.exec_time_ns is not None:
        LAST_EXEC_TIMES.append(res.exec_time_ns)
    return res.results


def _get_program(which, meta):
    key = (which, meta["epad"], tuple(meta["tchunks"]), meta.get("ngk"))
    if key not in _PROGRAMS:
        _PROGRAMS[key] = (_build_launch1 if which == 1 else _build_launch2)(meta)
    return _PROGRAMS[key]


def kernel(**inputs):
    global LAST_EXEC_TIMES
    LAST_EXEC_TIMES = []
    sim = os.environ.get("KGAT_SIM", "0") == "1"
    sim_cores = list(range(NCORES)) if sim else None

    x = np.asarray(inputs["x"], np.float32)
    ei = np.asarray(inputs["edge_index"], np.int64)
    batch = np.asarray(inputs["batch"], np.int64)
    edge_attr = np.asarray(inputs["edge_attr"], np.float32)

    W1ext, we1 = _prep_params(inputs["W1"], inputs["att_src1"], inputs["att_dst1"],
                              inputs["att_edge1"], inputs["We1"])
    W2ext, we2 = _prep_params(inputs["W2"], inputs["att_src2"], inputs["att_dst2"],
                              inputs["att_edge2"], inputs["We2"])
    b1 = np.asarray(inputs["b1"], np.float32)
    b2 = np.asarray(inputs["b2"], np.float32)
    lin_w = np.asarray(inputs["lin_w"], np.float32)
    lin_b = np.asarray(inputs["lin_b"], np.float32)

    cores, tchunks, epad = _prep_edges(ei[0], ei[1], edge_attr)
    pmats, g_lo, ngk = _prep_pool(batch)
    nchunk = epad // DT

    xpad = np.zeros((NPAD, F), np.float32)
    xpad[:N] = x
    xtc = np.ascontiguousarray(xpad.reshape(NCH_A, 128, F).transpose(0, 2, 1))
    iota8 = np.ascontiguousarray(
        np.tile(np.arange(128, dtype=np.float32), (128, BS)))

    meta = dict(epad=epad, nchunk=nchunk, tchunks=tchunks, ngk=ngk)

    iotap = np.arange(128, dtype=np.float32).reshape(128, 1).copy()

    def common_maps(k):
        ck = cores[k]
        return dict(esrc_w=ck["esrc_w"],
                    hxidx_w=ck["hxidx_w"], dstloc=ck["dstloc"],
                    dstlocT=ck["dstlocT"], eattrT=ck["eattrT"], iota8=iota8,
                    iotap=iotap)

    nc1 = _get_program(1, meta)
    in_maps1 = []
    for k in range(NCORES):
        m = common_maps(k)
        m.update(xtc=xtc, W1ext=W1ext, W2ext=W2ext, we=we1, b=b1)
        in_maps1.append(m)
    res1 = _run(nc1, in_maps1, sim_cores=sim_cores)

    h2full = np.concatenate([res1[k]["h2slice"][:NS] for k in range(NCORES)], 0)
    h2pad = np.zeros((NPAD, TW), np.float32)
    h2pad[:N] = h2full

    nc2 = _get_program(2, meta)
    in_maps2 = []
    for k in range(NCORES):
        m = common_maps(k)
        m.update(h2ext=h2pad, pmat=pmats[k], we=we2, b=b2,
                 lin_w=lin_w, lin_b=lin_b)
        in_maps2.append(m)
    res2 = _run(nc2, in_maps2, sim_cores=sim_cores)

    y = np.zeros((G, 2), np.float32)
    for k in range(NCORES):
        yk = res2[k]["yout"]
        for gi in range(ngk):
            g = g_lo[k] + gi
            if g < G:
                y[g] += yk[gi]
    return (y + lin_b[None, :]).astype(np.float32)


# revision 14
# speedup vs baseline: 1.2184x; 1.1439x over previous
"""Trainium2 Bass kernel for a 2-layer GAT (PyG GATConv, concat heads) +
global mean pool + linear head, distributed over 8 NeuronCores.

Strategy (self-contained; shapes hardcoded for this problem):
  - Destination-shard nodes across the 8 cores (2500 each); a core owns the
    edges whose destination lies in its slice. Graph pooling is shard-local.
  - he = edge_attr @ We is never materialized: its only use is
    a_e = he . att_e, which collapses to a_e = edge_attr @ we with the [4, 8]
    matrix we[d, h] = We[d, h*64:(h+1)*64] @ att_e[h] (host weight folding).
  - Self-loops (fill_value='mean') are folded analytically: the self-loop's
    a_e equals segment_sum(a_e)/max(deg, 1); both stats are accumulated as
    extra columns of the edge-aggregation matmul.
  - Softmax without max-subtraction: out_i = sum_e exp(r_e) h_src / sum exp(r_e)
    is mathematically identical to the max-normalized form (logits are O(10)).
  - Edge aggregation per 128-destination tile = PE matmul with the one-hot
    scatter matrix C[e, d] = (dst_local[e] == d), built on DVE from an iota
    constant; messages are dma_gather'ed source rows scaled by exp-logits.
    a_d[dst_e] is broadcast to edges with a second matmul using C^T (built
    from a partition-replicated dst-local row), avoiding a per-edge gather.
  - Node-table rows store features in bf16 (KGAT_BF16=1, default) with the
    16 attention logits (a_s|a_d) kept exact as f32 bytes inside the row;
    rows are 1280 B, satisfying dma_gather's 256 B-multiple constraint.
  - Two SPMD launches: launch 1 builds the layer-1 node table x @ W1ext,
    runs the layer-1 edge phase, and emits each core's slice of the layer-2
    table h2 = relu(out1) @ W2ext. The host concatenates slices (pure data
    movement) and launch 2 runs the layer-2 edge phase + pooling + head.

Graded entry point: kernel(**inputs) -> np.ndarray [200, 2] float32.
"""

import os
import numpy as np

# -------------------- problem constants --------------------
N, F, H, C, HC, ED, E, G = 20000, 128, 8, 64, 512, 4, 320000, 200
NEG = 0.2
NCORES = 8
NS = N // NCORES            # 2500 destination nodes per core
DT = 128                    # destinations per tile (PSUM partition dim)
NT = (NS + DT - 1) // DT    # 20 dst tiles per core
NPAD = ((N + 127) // 128) * 128   # 20096 table rows (pad rows zero)
NCH_A = NPAD // 128         # 157 node chunks in phase A
BS = 8                      # max 128-edge chunks per gather batch
# (dma_gather with num_idxs=2048 crashes the exec unit; 1024 is solid)

BF16 = os.environ.get("KGAT_BF16", "1") == "1"
# table row: [512 feat | 16 f32 logits (a_s|a_d) | pad]; bf16 rows are 640
# elems = 1280 B, f32 rows 576 elems = 2304 B (both % 256 B as required).
TBW = 640 if BF16 else 576

LAST_EXEC_TIMES = []        # ns per launch (for the test harness)
_PROGRAMS = {}


def _wrap16(a):
    """dma_gather index layout: idx j sits at [j % 16, j // 16] (int16),
    replicated across the 8 groups of 16 partitions."""
    a = np.asarray(a, np.int64)
    assert a.size % 16 == 0
    w = a.astype(np.int16).reshape(a.size // 16, 16).T
    return np.ascontiguousarray(np.tile(w, (8, 1)))


# ==================== host-side preprocessing ====================

def _prep_params(W, att_s, att_d, att_e, We):
    W = np.asarray(W, np.float32)
    att_s = np.asarray(att_s, np.float32)
    att_d = np.asarray(att_d, np.float32)
    att_e = np.asarray(att_e, np.float32)
    We = np.asarray(We, np.float32)
    As = np.zeros((HC, H), np.float32)
    Ad = np.zeros((HC, H), np.float32)
    for h in range(H):
        As[h * C:(h + 1) * C, h] = att_s[h]
        Ad[h * C:(h + 1) * C, h] = att_d[h]
    Wext = np.zeros((W.shape[0], HC + 16), np.float32)
    Wext[:, :HC] = W
    Wext[:, HC:HC + H] = W @ As
    Wext[:, HC + H:HC + 2 * H] = W @ Ad
    we = np.zeros((ED, H), np.float32)
    for h in range(H):
        we[:, h] = We[:, h * C:(h + 1) * C] @ att_e[h]
    return Wext, we


def _prep_edges(src, dst, edge_attr):
    counts = np.zeros((NCORES, NT), np.int64)
    per_core = []
    for k in range(NCORES):
        m = (dst >= k * NS) & (dst < (k + 1) * NS)
        s, d, ea = src[m], dst[m], edge_attr[m]
        dloc = d - k * NS
        t = dloc // DT
        order = np.argsort(t, kind="stable")
        s, dloc, ea = s[order], dloc[order], ea[order]
        counts[k] = np.bincount(t[order], minlength=NT)
        per_core.append((s, dloc, ea))
    tchunks = np.maximum(1, (counts.max(axis=0) + DT - 1) // DT).astype(np.int64)
    epad = int(tchunks.sum()) * DT
    nchunk = epad // DT
    cores = []
    for k in range(NCORES):
        s, dloc, ea = per_core[k]
        esrc = np.zeros(epad, np.int64)
        dlocf = np.full(epad, 999.0, np.float32)
        eat = np.zeros((epad, ED), np.float32)
        off = pos = 0
        for ti in range(NT):
            n = int(counts[k, ti])
            sl = slice(pos, pos + n)
            esrc[off:off + n] = s[sl]
            dlocf[off:off + n] = (dloc[sl] - ti * DT).astype(np.float32)
            eat[off:off + n] = ea[sl]
            off += int(tchunks[ti]) * DT
            pos += n
        cores.append(dict(
            esrc_w=_wrap16(esrc),
            dstloc=np.ascontiguousarray(dlocf.reshape(nchunk, DT).T),
            dstlocT=np.ascontiguousarray(dlocf.reshape(nchunk, DT)),
            eattrT=np.ascontiguousarray(eat.T),
            hxidx_w=_wrap16(np.arange(k * NS, k * NS + NT * DT)),
        ))
    return cores, tchunks, epad


def _prep_pool(batch):
    """Per-core pooling matrix over the core's contiguous graph-id window,
    with 1/count baked in. Host overlap-adds windows afterwards (pure unshard
    glue; disjoint for the reference's uniform batch)."""
    batch = np.asarray(batch, np.int64)
    cnt = np.bincount(batch, minlength=G).astype(np.float32)
    g_lo = [int(batch[k * NS:(k + 1) * NS].min()) for k in range(NCORES)]
    g_hi = [int(batch[k * NS:(k + 1) * NS].max()) for k in range(NCORES)]
    ngk = min(max(max(h - l + 1 for l, h in zip(g_lo, g_hi)), 1), 128)
    pmats = []
    for k in range(NCORES):
        Pm = np.zeros((NT * DT, ngk), np.float32)
        bk = batch[k * NS:(k + 1) * NS]
        gl = np.clip(bk - g_lo[k], 0, ngk - 1)
        Pm[np.arange(NS), gl] = 1.0 / np.maximum(cnt[bk], 1.0)
        pm_dev = Pm.reshape(NT, DT, ngk).transpose(1, 0, 2).reshape(DT, NT * ngk)
        pmats.append(np.ascontiguousarray(pm_dev))
    return pmats, g_lo, ngk


# ==================== Bass program builders ====================

def _common_inputs(nc, mybir, epad, nchunk):
    f32, i16 = mybir.dt.float32, mybir.dt.int16
    t = {}
    t["esrc_w"] = nc.dram_tensor("esrc_w", [128, epad // 16], i16, kind="ExternalInput")
    t["hxidx_w"] = nc.dram_tensor("hxidx_w", [128, NT * DT // 16], i16, kind="ExternalInput")
    t["dstloc"] = nc.dram_tensor("dstloc", [128, nchunk], f32, kind="ExternalInput")
    t["dstlocT"] = nc.dram_tensor("dstlocT", [nchunk, 128], f32, kind="ExternalInput")
    t["iotap"] = nc.dram_tensor("iotap", [128, 1], f32, kind="ExternalInput")
    t["eattrT"] = nc.dram_tensor("eattrT", [4, epad], f32, kind="ExternalInput")
    t["iota8"] = nc.dram_tensor("iota8", [128, BS * 128], f32, kind="ExternalInput")
    t["we"] = nc.dram_tensor("we", [4, H], f32, kind="ExternalInput")
    t["b"] = nc.dram_tensor("b", [HC], f32, kind="ExternalInput")
    return t


def _load_common_sbuf(nc, tc, ctx, mybir, t):
    f32, i16 = mybir.dt.float32, mybir.dt.int16
    cpool = ctx.enter_context(tc.tile_pool(name="const", bufs=1))
    sb = {}
    ew = t["esrc_w"].shape[1]
    sb["esrc"] = cpool.tile([128, ew], i16, tag="esrc", name="esrc_sb")
    nc.sync.dma_start(out=sb["esrc"][:], in_=t["esrc_w"].ap())
    sb["hxidx"] = cpool.tile([128, NT * DT // 16], i16, tag="hxidx", name="hxidx_sb")
    nc.sync.dma_start(out=sb["hxidx"][:], in_=t["hxidx_w"].ap())
    sb["iota"] = cpool.tile([128, BS, 128], f32, tag="iota", name="iota_sb")
    nc.sync.dma_start(out=sb["iota"][:],
                      in_=t["iota8"].ap().rearrange("p (c x) -> p c x", c=BS))
    sb["web"] = cpool.tile([4, H], f32, tag="web", name="web_sb")
    nc.sync.dma_start(out=sb["web"][:], in_=t["we"].ap())
    sb["iotap"] = cpool.tile([128, 1], f32, tag="iotap", name="iotap_sb")
    nc.sync.dma_start(out=sb["iotap"][:], in_=t["iotap"].ap())
    sb["b"] = cpool.tile([128, HC], f32, tag="bb", name="b_sb")
    nc.sync.dma_start(out=sb["b"][:], in_=t["b"].ap()[None, :].to_broadcast([128, HC]))
    return sb, cpool


def _logit_view(ap3, mybir):
    """[P, n, TBW] table-row tile -> [P, n, 16] f32 view of the logit block."""
    if BF16:
        return ap3[:, :, HC:HC + 32].bitcast(mybir.dt.float32)
    return ap3[:, :, HC:HC + 16]


def _edge_phase(nc, mybir, ctx, tc, cfg, tbl, sb, relu, pool_cfg=None):
    """Per-layer edge phase + per-tile epilogue. Stores normalized tiles to
    cfg['store'] (if set) and/or accumulates graph pooling (pool_cfg)."""
    tchunks = cfg["tchunks"]
    f32 = mybir.dt.float32
    td = mybir.dt.bfloat16 if BF16 else f32

    gpool = ctx.enter_context(tc.tile_pool(name="gpool", bufs=2))
    spool = ctx.enter_context(tc.tile_pool(name="spool", bufs=2))
    ps_feat_p = ctx.enter_context(tc.tile_pool(name="psf", bufs=2, space="PSUM"))
    ps_stat_p = ctx.enter_context(tc.tile_pool(name="pss", bufs=2, space="PSUM"))
    ps_ae_p = ctx.enter_context(tc.tile_pool(name="psa", bufs=2, space="PSUM"))
    if pool_cfg is not None:
        ps_pool_p = ctx.enter_context(tc.tile_pool(name="psp", bufs=1, space="PSUM"))
        pool_ps = ps_pool_p.tile([pool_cfg["ngk"], 2], f32, tag="poolps", name="pool_ps")

    tbl_ap = tbl.ap()

    chunk0 = 0
    for ti in range(NT):
        nch = int(tchunks[ti])
        ps_feat = ps_feat_p.tile([DT, HC], f32, tag="feat")
        ps_stat = ps_stat_p.tile([DT, 17], f32, tag="stat")
        # local table rows for this dst tile; the f32 a_d block doubles as the
        # rhs of the per-chunk a_d broadcast matmul
        hx = spool.tile([DT, 1, TBW], td, tag="hx")
        nc.gpsimd.dma_gather(hx[:], tbl_ap,
                             sb["hxidx"][:, ti * 8:(ti + 1) * 8],
                             DT, DT, TBW)
        hxf = _logit_view(hx[:], mybir)        # [DT, 1, 16] f32
        first_chunk = True
        done = 0
        while done < nch:
            bs = min(BS, nch - done)
            c0 = chunk0 + done
            nidx = bs * DT
            hg = gpool.tile([DT, bs, TBW], td, tag="hg")
            nc.gpsimd.dma_gather(hg[:], tbl_ap,
                                 sb["esrc"][:, c0 * 8: c0 * 8 + nidx // 16],
                                 nidx, nidx, TBW)
            hgf = _logit_view(hg[:], mybir)    # [DT, bs, 16] f32
            eat = spool.tile([4, bs * DT], f32, tag="eat")
            nc.sync.dma_start(out=eat[:],
                              in_=cfg["eattrT"].ap()[:, c0 * DT:(c0 + bs) * DT])
            dl = spool.tile([DT, bs], f32, tag="dl")
            nc.sync.dma_start(out=dl[:], in_=cfg["dstloc"].ap()[:, c0:c0 + bs])
            # dst-locals replicated across partitions for the transposed
            # scatter matrix (the partition broadcast comes from the DMA)
            dlb = spool.tile([DT, bs, DT], f32, tag="dlb")
            nc.sync.dma_start(
                out=dlb[:],
                in_=cfg["dstlocT"].ap()[None, c0:c0 + bs, :].to_broadcast(
                    [DT, bs, DT]))
            ct = spool.tile([DT, bs, DT], f32, tag="ct")
            nc.vector.tensor_scalar(ct[:], dlb[:], sb["iotap"][:], None,
                                    mybir.AluOpType.is_equal)

            ps_ae = ps_ae_p.tile([DT, bs, 16], f32, tag="ae")
            for c in range(bs):
                nc.tensor.matmul(out=ps_ae[:, c, 0:H],
                                 lhsT=eat[:, c * DT:(c + 1) * DT],
                                 rhs=sb["web"][:], start=True, stop=True)
                # a_d[dst_e] broadcast: CT.T @ a_d_tile
                nc.tensor.matmul(out=ps_ae[:, c, H:2 * H],
                                 lhsT=ct[:, c, :],
                                 rhs=hxf[:, 0, 8:16],
                                 start=True, stop=True)
            s = spool.tile([DT, bs, H], f32, tag="s")
            nc.vector.tensor_tensor(out=s[:], in0=hgf[:, :, 0:H],
                                    in1=ps_ae[:, :, H:2 * H],
                                    op=mybir.AluOpType.add)
            nc.vector.tensor_tensor(out=s[:], in0=s[:], in1=ps_ae[:, :, 0:H],
                                    op=mybir.AluOpType.add)
            s2 = spool.tile([DT, bs, H], f32, tag="s2")
            nc.vector.tensor_scalar_mul(s2[:], s[:], NEG)
            nc.vector.tensor_tensor(out=s[:], in0=s[:], in1=s2[:],
                                    op=mybir.AluOpType.max)
            wae = spool.tile([DT, bs, 17], td, tag="wae")
            nc.scalar.activation(wae[:, :, 0:H], s[:],
                                 mybir.ActivationFunctionType.Exp)
            nc.vector.tensor_copy(out=wae[:, :, H:2 * H], in_=ps_ae[:, :, 0:H])
            nc.vector.memset(wae[:, :, 2 * H:2 * H + 1], 1.0)
            cm = spool.tile([DT, bs, DT], td, tag="cm")
            nc.vector.tensor_tensor(
                out=cm[:], in0=dl[:, :, None].to_broadcast([DT, bs, DT]),
                in1=sb["iota"][:, 0:bs, :], op=mybir.AluOpType.is_equal)
            hg4 = hg[:, :, 0:HC].rearrange("p c (h x) -> p c h x", h=H)
            nc.vector.tensor_tensor(
                out=hg4, in0=hg4,
                in1=wae[:, :, 0:H].to_broadcast([DT, bs, H, C]),
                op=mybir.AluOpType.mult)
            for c in range(bs):
                last = (done + c == nch - 1)
                nc.tensor.matmul(out=ps_feat[:], lhsT=cm[:, c, :],
                                 rhs=hg[:, c, 0:HC],
                                 start=first_chunk, stop=last,
                                 skip_group_check=True)
                nc.tensor.matmul(out=ps_stat[:], lhsT=cm[:, c, :],
                                 rhs=wae[:, c, :],
                                 start=first_chunk, stop=last,
                                 skip_group_check=True)
                first_chunk = False
            done += bs
        chunk0 += nch

        # ---- per-tile epilogue ----
        dmax = spool.tile([DT, 1], f32, tag="dmax")
        nc.vector.tensor_scalar_max(dmax[:], ps_stat[:, 16:17], 1.0)
        rdeg = spool.tile([DT, 1], f32, tag="rdeg")
        nc.vector.reciprocal(rdeg[:], dmax[:])
        sl_ = spool.tile([DT, H], f32, tag="sl")
        nc.vector.tensor_tensor(out=sl_[:], in0=ps_stat[:, 8:16],
                                in1=rdeg[:].to_broadcast([DT, H]),
                                op=mybir.AluOpType.mult)
        nc.vector.tensor_tensor(out=sl_[:], in0=sl_[:], in1=hxf[:, 0, 0:8],
                                op=mybir.AluOpType.add)
        nc.vector.tensor_tensor(out=sl_[:], in0=sl_[:], in1=hxf[:, 0, 8:16],
                                op=mybir.AluOpType.add)
        sl2 = spool.tile([DT, H], f32, tag="sl2")
        nc.vector.tensor_scalar_mul(sl2[:], sl_[:], NEG)
        nc.vector.tensor_tensor(out=sl_[:], in0=sl_[:], in1=sl2[:],
                                op=mybir.AluOpType.max)
        wl = spool.tile([DT, H], f32, tag="wl")
        nc.scalar.activation(wl[:], sl_[:], mybir.ActivationFunctionType.Exp)
        den = spool.tile([DT, H], f32, tag="den")
        nc.vector.tensor_tensor(out=den[:], in0=ps_stat[:, 0:8], in1=wl[:],
                                op=mybir.AluOpType.add)
        rden = spool.tile([DT, H], f32, tag="rden")
        nc.vector.reciprocal(rden[:], den[:])
        out_t = spool.tile([DT, HC], f32, tag="outt")
        o4 = out_t[:].rearrange("p (h x) -> p h x", h=H)
        nc.vector.tensor_tensor(
            out=o4, in0=hx[:, 0, 0:HC].rearrange("p (h x) -> p h x", h=H),
            in1=wl[:, :, None].to_broadcast([DT, H, C]), op=mybir.AluOpType.mult)
        nc.vector.tensor_tensor(out=out_t[:], in0=out_t[:], in1=ps_feat[:],
                                op=mybir.AluOpType.add)
        nc.vector.tensor_tensor(
            out=o4, in0=o4, in1=rden[:, :, None].to_broadcast([DT, H, C]),
            op=mybir.AluOpType.mult)
        nc.vector.tensor_tensor(out=out_t[:], in0=out_t[:], in1=sb["b"][:],
                                op=mybir.AluOpType.add)
        if relu:
            nc.vector.tensor_scalar_max(out_t[:], out_t[:], 0.0)
        if cfg.get("store") is not None:
            nc.sync.dma_start(out=cfg["store"].ap()[ti * DT:(ti + 1) * DT, :],
                              in_=out_t[:])
        if pool_cfg is not None:
            ngk = pool_cfg["ngk"]
            ytile = spool.tile([DT, 2], f32, tag="yt")
            tmp = spool.tile([DT, HC], f32, tag="ytmp")
            for j in range(2):
                nc.vector.tensor_tensor(out=tmp[:], in0=out_t[:],
                                        in1=sb["linw%d" % j][:],
                                        op=mybir.AluOpType.mult)
                nc.vector.tensor_reduce(out=ytile[:, j:j + 1], in_=tmp[:],
                                        axis=mybir.AxisListType.X,
                                        op=mybir.AluOpType.add)
            nc.tensor.matmul(out=pool_ps[:],
                             lhsT=sb["pmat"][:, ti * ngk:(ti + 1) * ngk],
                             rhs=ytile[:], start=(ti == 0), stop=(ti == NT - 1),
                             skip_group_check=True)

    if pool_cfg is not None:
        ysb = spool.tile([pool_cfg["ngk"], 2], f32, tag="ysb")
        nc.vector.tensor_tensor(out=ysb[:], in0=pool_ps[:], in1=sb["linb"][:],
                                op=mybir.AluOpType.add)
        nc.sync.dma_start(out=pool_cfg["yout"].ap()[:, :], in_=ysb[:])


def _store_table_rows(nc, mybir, dst_tile, c, psF, psL):
    """Write one node chunk into a [128, nb, TBW] table tile: features
    (cast to the table dtype) + the 16 f32 logits kept bit-exact."""
    f32 = mybir.dt.float32
    nc.vector.tensor_copy(out=dst_tile[:, c, 0:HC], in_=psF[:])
    if BF16:
        nc.vector.tensor_copy(
            out=dst_tile[:, c, HC:HC + 32].bitcast(f32), in_=psL[:])
    else:
        nc.vector.tensor_copy(out=dst_tile[:, c, HC:HC + 16], in_=psL[:])


def _build_launch1(meta):
    import concourse.bacc as bacc
    import concourse.tile as tile
    from concourse import mybir
    from contextlib import ExitStack
    from concourse.masks import make_identity

    f32 = mybir.dt.float32
    td = mybir.dt.bfloat16 if BF16 else f32
    epad, nchunk, tchunks = meta["epad"], meta["nchunk"], meta["tchunks"]

    nc = bacc.Bacc("TRN2", target_bir_lowering=False, debug=False)
    t = _common_inputs(nc, mybir, epad, nchunk)
    t["xtc"] = nc.dram_tensor("xtc", [NCH_A, 128, 128], f32, kind="ExternalInput")
    t["W1ext"] = nc.dram_tensor("W1ext", [128, HC + 16], f32, kind="ExternalInput")
    t["W2ext"] = nc.dram_tensor("W2ext", [512, HC + 16], f32, kind="ExternalInput")
    tbl = nc.dram_tensor("h1ext", [NPAD, TBW], td)
    x1out = nc.dram_tensor("x1out", [NT * DT, HC], f32)
    h2slice = nc.dram_tensor("h2slice", [NT * DT, TBW], td, kind="ExternalOutput")
    parts = os.environ.get("KGAT_L1_PARTS", "AEC")

    with tile.TileContext(nc) as tc, ExitStack() as ctx:
        sb, cpool = _load_common_sbuf(nc, tc, ctx, mybir, t)
        w1sb = cpool.tile([128, HC + 16], f32, tag="w1sb")
        nc.sync.dma_start(out=w1sb[:], in_=t["W1ext"].ap())
        w2sb = []
        for kc in range(4):
            w = cpool.tile([128, HC + 16], f32, tag="w2sb%d" % kc, name="w2sb%d" % kc)
            nc.sync.dma_start(out=w[:], in_=t["W2ext"].ap()[kc * 128:(kc + 1) * 128, :])
            w2sb.append(w)
        ident = cpool.tile([128, 128], f32, tag="ident")
        make_identity(nc, ident[:])

        # ---------------- phase A: h1ext = xpad @ W1ext ----------------
        with tc.tile_pool(name="pA", bufs=3) as pA, \
             tc.tile_pool(name="psA", bufs=2, space="PSUM") as psA_p, \
             tc.tile_pool(name="psA2", bufs=2, space="PSUM") as psA2_p:
            AB = 8
            for t0 in range(0, NCH_A, AB):
                nb = min(AB, NCH_A - t0)
                xb = pA.tile([128, nb, 128], f32, tag="xb")
                nc.sync.dma_start(
                    out=xb[:],
                    in_=t["xtc"].ap()[t0:t0 + nb].rearrange("c f n -> f c n"))
                hb = pA.tile([128, nb, TBW], td, tag="hb")
                if BF16:
                    nc.vector.memset(hb[:, :, HC + 32:TBW], 0.0)
                else:
                    nc.vector.memset(hb[:, :, HC + 16:TBW], 0.0)
                for c in range(nb):
                    psA = psA_p.tile([128, HC], f32, tag="psA")
                    psA2 = psA2_p.tile([128, 16], f32, tag="psA2")
                    nc.tensor.matmul(out=psA[:], lhsT=xb[:, c, :],
                                     rhs=w1sb[:, 0:HC], start=True, stop=True)
                    nc.tensor.matmul(out=psA2[:], lhsT=xb[:, c, :],
                                     rhs=w1sb[:, HC:HC + 16], start=True, stop=True)
                    _store_table_rows(nc, mybir, hb, c, psA, psA2)
                nc.sync.dma_start(
                    out=tbl.ap()[t0 * 128:(t0 + nb) * 128, :].rearrange(
                        "(c p) w -> p c w", p=128),
                    in_=hb[:])
        tc.strict_bb_all_engine_barrier()

        # ---------------- layer-1 edge phase ----------------
        if "E" in parts:
            cfg = dict(tchunks=tchunks, eattrT=t["eattrT"], dstloc=t["dstloc"],
                       dstlocT=t["dstlocT"], store=x1out)
            with ExitStack() as ectx:
                _edge_phase(nc, mybir, ectx, tc, cfg, tbl, sb, relu=True)
            tc.strict_bb_all_engine_barrier()

        # ---------------- phase C: h2slice = x1out @ W2ext ----------------
        if "C" in parts:
            with tc.tile_pool(name="pC", bufs=2) as pC, \
                 tc.tile_pool(name="psTr", bufs=2, space="PSUM") as psTr_p, \
                 tc.tile_pool(name="psH", bufs=2, space="PSUM") as psH_p, \
                 tc.tile_pool(name="psH2", bufs=2, space="PSUM") as psH2_p:
                for ti in range(NT):
                    x2 = pC.tile([128, HC], f32, tag="x2")
                    nc.sync.dma_start(out=x2[:],
                                      in_=x1out.ap()[ti * DT:(ti + 1) * DT, :])
                    x2t = pC.tile([128, HC], f32, tag="x2t")
                    for kc in range(4):
                        ptr = psTr_p.tile([128, 128], f32, tag="ptr")
                        nc.tensor.transpose(out=ptr[:],
                                            in_=x2[:, kc * 128:(kc + 1) * 128],
                                            identity=ident[:])
                        nc.vector.tensor_copy(out=x2t[:, kc * 128:(kc + 1) * 128],
                                              in_=ptr[:])
                    psH = psH_p.tile([128, HC], f32, tag="psH")
                    psH2 = psH2_p.tile([128, 16], f32, tag="psH2")
                    for kc in range(4):
                        nc.tensor.matmul(out=psH[:],
                                         lhsT=x2t[:, kc * 128:(kc + 1) * 128],
                                         rhs=w2sb[kc][:, 0:HC],
                                         start=(kc == 0), stop=(kc == 3))
                        nc.tensor.matmul(out=psH2[:],
                                         lhsT=x2t[:, kc * 128:(kc + 1) * 128],
                                         rhs=w2sb[kc][:, HC:HC + 16],
                                         start=(kc == 0), stop=(kc == 3))
                    h2t = pC.tile([128, 1, TBW], td, tag="h2t")
                    if BF16:
                        nc.vector.memset(h2t[:, :, HC + 32:TBW], 0.0)
                    else:
                        nc.vector.memset(h2t[:, :, HC + 16:TBW], 0.0)
                    _store_table_rows(nc, mybir, h2t, 0, psH, psH2)
                    nc.sync.dma_start(out=h2slice.ap()[ti * DT:(ti + 1) * DT, :],
                                      in_=h2t[:, 0, :])

    nc.compile()
    return nc


def _build_launch2(meta):
    import concourse.bacc as bacc
    import concourse.tile as tile
    from concourse import mybir
    from contextlib import ExitStack

    f32 = mybir.dt.float32
    td = mybir.dt.bfloat16 if BF16 else f32
    epad, nchunk, tchunks = meta["epad"], meta["nchunk"], meta["tchunks"]
    ngk = meta["ngk"]

    nc = bacc.Bacc("TRN2", target_bir_lowering=False, debug=False)
    t = _common_inputs(nc, mybir, epad, nchunk)
    t["h2ext"] = nc.dram_tensor("h2ext", [NPAD, TBW], td, kind="ExternalInput")
    t["pmat"] = nc.dram_tensor("pmat", [128, NT * ngk], f32, kind="ExternalInput")
    t["lin_w"] = nc.dram_tensor("lin_w", [HC, 2], f32, kind="ExternalInput")
    t["lin_b"] = nc.dram_tensor("lin_b", [2], f32, kind="ExternalInput")
    yout = nc.dram_tensor("yout", [ngk, 2], f32, kind="ExternalOutput")

    with tile.TileContext(nc) as tc, ExitStack() as ctx:
        sb, cpool = _load_common_sbuf(nc, tc, ctx, mybir, t)
        sb["pmat"] = cpool.tile([128, NT * ngk], f32, tag="pm", name="pm_sb")
        nc.sync.dma_start(out=sb["pmat"][:], in_=t["pmat"].ap())
        for j in range(2):
            lw = cpool.tile([128, HC], f32, tag="linw%d" % j, name="linw%d_sb" % j)
            nc.sync.dma_start(
                out=lw[:],
                in_=t["lin_w"].ap()[:, j:j + 1].rearrange("a b -> b a").to_broadcast([128, HC]))
            sb["linw%d" % j] = lw
        sb["linb"] = cpool.tile([ngk, 2], f32, tag="linb", name="linb_sb")
        nc.sync.dma_start(out=sb["linb"][:],
                          in_=t["lin_b"].ap()[None, :].to_broadcast([ngk, 2]))

        cfg = dict(tchunks=tchunks, eattrT=t["eattrT"], dstloc=t["dstloc"],
                   dstlocT=t["dstlocT"], store=None)
        pool_cfg = dict(ngk=ngk, yout=yout)
        with ExitStack() as ectx:
            _edge_phase(nc, mybir, ectx, tc, cfg, t["h2ext"], sb, relu=False,
                        pool_cfg=pool_cfg)

    nc.compile()
    return nc


# ==================== driver ====================

def _install_trace_shim():
    """Dev-only (KGAT_TRACE=1): register the axon NTFF profile hook that this
    image's antenv lacks, and keep profile artifacts local."""
    import sys, types
    try:
        from antenv import axon_hooks  # noqa: F401
        return
    except ImportError:
        pass
    try:
        from trn_agent_boot.trn_boot import _ntff_profile_via_ctypes
        mod = types.ModuleType("antenv.axon_hooks")
        mod._h = _ntff_profile_via_ctypes("/opt/axon/libaxon_pjrt.so")
        mod.set_axon_ntff_profile_hook = lambda h: setattr(mod, "_h", h)
        mod.get_axon_ntff_profile_hook = lambda: mod._h
        sys.modules["antenv.axon_hooks"] = mod
        import antenv
        antenv.axon_hooks = mod
        import concourse.bass_utils as bu
        bu.upload_artifacts = lambda d: d
    except Exception as e:  # pragma: no cover
        print(f"trace shim failed ({e}); falling back to untraced run")


def _run(nc, in_maps, sim_cores=None):
    global LAST_EXEC_TIMES
    if sim_cores is not None:
        from concourse.bass_interp import CoreSim
        out_names = [a.memorylocations[0].name
                     for a in nc.m.functions[0].allocations
                     if getattr(a, "kind", None) == "ExternalOutput"]
        outs = [None] * len(in_maps)
        for ci in sim_cores:
            # f32 logit bytes inside bf16 table rows false-positive the
            # simulator's NaN scan; disable it (bit-exact execution unchanged)
            s = CoreSim(nc, trace=False, require_finite=False, require_nnan=False)
            for k, v in in_maps[ci].items():
                s.tensor(k)[:] = v
            s.simulate(check_with_hw=False)
            outs[ci] = {n: np.array(s.tensor(n)) for n in out_names}
        return outs
    trace = os.environ.get("KGAT_TRACE", "0") == "1"
    if trace:
        _install_trace_shim()
    from concourse.bass_utils import run_bass_kernel_spmd
    import time as _time
    t0 = _time.time()
    res = run_bass_kernel_spmd(nc, in_maps, list(range(NCORES)), trace=trace)
    if os.environ.get("KGAT_VERBOSE", "0") == "1":
        print(f"launch wall {_time.time() - t0:.2f}s exec_ns={res.exec_time_ns}")
    if res.exec_time_ns is not None:
        LAST_EXEC_TIMES.append(res.exec_time_ns)
    return res.results


def _get_program(which, meta):
    key = (which, BF16, meta["epad"], tuple(meta["tchunks"]), meta.get("ngk"))
    if key not in _PROGRAMS:
        _PROGRAMS[key] = (_build_launch1 if which == 1 else _build_launch2)(meta)
    return _PROGRAMS[key]


def kernel(**inputs):
    global LAST_EXEC_TIMES
    LAST_EXEC_TIMES = []
    sim = os.environ.get("KGAT_SIM", "0") == "1"
    sim_cores = list(range(NCORES)) if sim else None

    x = np.asarray(inputs["x"], np.float32)
    ei = np.asarray(inputs["edge_index"], np.int64)
    batch = np.asarray(inputs["batch"], np.int64)
    edge_attr = np.asarray(inputs["edge_attr"], np.float32)

    W1ext, we1 = _prep_params(inputs["W1"], inputs["att_src1"], inputs["att_dst1"],
                              inputs["att_edge1"], inputs["We1"])
    W2ext, we2 = _prep_params(inputs["W2"], inputs["att_src2"], inputs["att_dst2"],
                              inputs["att_edge2"], inputs["We2"])
    b1 = np.asarray(inputs["b1"], np.float32)
    b2 = np.asarray(inputs["b2"], np.float32)
    lin_w = np.asarray(inputs["lin_w"], np.float32)
    lin_b = np.asarray(inputs["lin_b"], np.float32)

    cores, tchunks, epad = _prep_edges(ei[0], ei[1], edge_attr)
    pmats, g_lo, ngk = _prep_pool(batch)
    nchunk = epad // DT

    xpad = np.zeros((NPAD, F), np.float32)
    xpad[:N] = x
    xtc = np.ascontiguousarray(xpad.reshape(NCH_A, 128, F).transpose(0, 2, 1))
    iota8 = np.ascontiguousarray(
        np.tile(np.arange(128, dtype=np.float32), (128, BS)))
    iotap = np.arange(128, dtype=np.float32).reshape(128, 1).copy()

    meta = dict(epad=epad, nchunk=nchunk, tchunks=tchunks, ngk=ngk)

    def common_maps(k):
        ck = cores[k]
        return dict(esrc_w=ck["esrc_w"], hxidx_w=ck["hxidx_w"],
                    dstloc=ck["dstloc"], dstlocT=ck["dstlocT"],
                    eattrT=ck["eattrT"], iota8=iota8, iotap=iotap)

    nc1 = _get_program(1, meta)
    in_maps1 = []
    for k in range(NCORES):
        m = common_maps(k)
        m.update(xtc=xtc, W1ext=W1ext, W2ext=W2ext, we=we1, b=b1)
        in_maps1.append(m)
    res1 = _run(nc1, in_maps1, sim_cores=sim_cores)

    h2full = np.concatenate([res1[k]["h2slice"][:NS] for k in range(NCORES)], 0)
    h2pad = np.zeros((NPAD, TBW), h2full.dtype)
    h2pad[:N] = h2full

    nc2 = _get_program(2, meta)
    in_maps2 = []
    for k in range(NCORES):
        m = common_maps(k)
        m.update(h2ext=h2pad, pmat=pmats[k], we=we2, b=b2,
                 lin_w=lin_w, lin_b=lin_b)
        in_maps2.append(m)
    res2 = _run(nc2, in_maps2, sim_cores=sim_cores)

    y = np.zeros((G, 2), np.float32)
    for k in range(NCORES):
        yk = np.asarray(res2[k]["yout"], np.float32)
        for gi in range(ngk):
            g = g_lo[k] + gi
            if g < G:
                y[g] += yk[gi]
    return (y + lin_b[None, :]).astype(np.float32)


# revision 16
# speedup vs baseline: 1.4325x; 1.1758x over previous
"""Trainium2 Bass kernel for a 2-layer GAT (PyG GATConv, concat heads) +
global mean pool + linear head, distributed over 8 NeuronCores.

Strategy (self-contained; shapes hardcoded for this problem):
  - Destination-shard nodes across the 8 cores (2500 each); a core owns the
    edges whose destination lies in its slice. Graph pooling is shard-local.
  - he = edge_attr @ We is never materialized: its only use is
    a_e = he . att_e, which collapses to a_e = edge_attr @ we with the [4, 8]
    matrix we[d, h] = We[d, h*64:(h+1)*64] @ att_e[h] (host weight folding).
  - Self-loops (fill_value='mean') are folded analytically: the self-loop's
    a_e equals segment_sum(a_e)/max(deg, 1); both stats are accumulated as
    extra columns of the edge-aggregation matmul.
  - Softmax without max-subtraction: out_i = sum_e exp(r_e) h_src / sum exp(r_e)
    is mathematically identical to the max-normalized form (logits are O(10)).
  - Edge aggregation per 128-destination tile = PE matmul with the one-hot
    scatter matrix C[e, d] = (dst_local[e] == d), built on DVE from an iota
    constant; messages are dma_gather'ed source rows scaled by exp-logits.
    a_d[dst_e] is broadcast to edges with a second matmul using C^T (built
    from a partition-replicated dst-local row), avoiding a per-edge gather.
  - Node-table rows store features in bf16 (KGAT_BF16=1, default) with the
    16 attention logits (a_s|a_d) kept exact as f32 bytes inside the row;
    rows are 1280 B, satisfying dma_gather's 256 B-multiple constraint.
  - Two SPMD launches: launch 1 builds the layer-1 node table x @ W1ext,
    runs the layer-1 edge phase, and emits each core's slice of the layer-2
    table h2 = relu(out1) @ W2ext. The host concatenates slices (pure data
    movement) and launch 2 runs the layer-2 edge phase + pooling + head.

Graded entry point: kernel(**inputs) -> np.ndarray [200, 2] float32.
"""

import os
import numpy as np

# -------------------- problem constants --------------------
N, F, H, C, HC, ED, E, G = 20000, 128, 8, 64, 512, 4, 320000, 200
NEG = 0.2
NCORES = 8
NS = N // NCORES            # 2500 destination nodes per core
DT = 128                    # destinations per tile (PSUM partition dim)
NT = (NS + DT - 1) // DT    # 20 dst tiles per core
NPAD = ((N + 127) // 128) * 128   # 20096 table rows (pad rows zero)
NCH_A = NPAD // 128         # 157 node chunks in phase A
BS = 8                      # max 128-edge chunks per gather batch
QN = int(os.environ.get("KGAT_QN", "1"))   # SWDGE queues to rotate gathers over
# (dma_gather with num_idxs=2048 crashes the exec unit; 1024 is solid)

BF16 = os.environ.get("KGAT_BF16", "1") == "1"
# table row: [512 feat | 16 f32 logits (a_s|a_d) | pad]; bf16 rows are 640
# elems = 1280 B, f32 rows 576 elems = 2304 B (both % 256 B as required).
TBW = 640 if BF16 else 576

LAST_EXEC_TIMES = []        # ns per launch (for the test harness)
_PROGRAMS = {}


def _wrap16(a):
    """dma_gather index layout: idx j sits at [j % 16, j // 16] (int16),
    replicated across the 8 groups of 16 partitions."""
    a = np.asarray(a, np.int64)
    assert a.size % 16 == 0
    w = a.astype(np.int16).reshape(a.size // 16, 16).T
    return np.ascontiguousarray(np.tile(w, (8, 1)))


# ==================== host-side preprocessing ====================

def _prep_params(W, att_s, att_d, att_e, We):
    W = np.asarray(W, np.float32)
    att_s = np.asarray(att_s, np.float32)
    att_d = np.asarray(att_d, np.float32)
    att_e = np.asarray(att_e, np.float32)
    We = np.asarray(We, np.float32)
    As = np.zeros((HC, H), np.float32)
    Ad = np.zeros((HC, H), np.float32)
    for h in range(H):
        As[h * C:(h + 1) * C, h] = att_s[h]
        Ad[h * C:(h + 1) * C, h] = att_d[h]
    Wext = np.zeros((W.shape[0], HC + 16), np.float32)
    Wext[:, :HC] = W
    Wext[:, HC:HC + H] = W @ As
    Wext[:, HC + H:HC + 2 * H] = W @ Ad
    we = np.zeros((ED, H), np.float32)
    for h in range(H):
        we[:, h] = We[:, h * C:(h + 1) * C] @ att_e[h]
    return Wext, we


def _prep_edges(src, dst, edge_attr):
    counts = np.zeros((NCORES, NT), np.int64)
    per_core = []
    for k in range(NCORES):
        m = (dst >= k * NS) & (dst < (k + 1) * NS)
        s, d, ea = src[m], dst[m], edge_attr[m]
        dloc = d - k * NS
        t = dloc // DT
        order = np.argsort(t, kind="stable")
        s, dloc, ea = s[order], dloc[order], ea[order]
        counts[k] = np.bincount(t[order], minlength=NT)
        per_core.append((s, dloc, ea))
    tchunks = np.maximum(1, (counts.max(axis=0) + DT - 1) // DT).astype(np.int64)
    epad = int(tchunks.sum()) * DT
    nchunk = epad // DT
    cores = []
    for k in range(NCORES):
        s, dloc, ea = per_core[k]
        esrc = np.zeros(epad, np.int64)
        dlocf = np.full(epad, 999.0, np.float32)
        eat = np.zeros((epad, ED), np.float32)
        off = pos = 0
        for ti in range(NT):
            n = int(counts[k, ti])
            sl = slice(pos, pos + n)
            esrc[off:off + n] = s[sl]
            dlocf[off:off + n] = (dloc[sl] - ti * DT).astype(np.float32)
            eat[off:off + n] = ea[sl]
            off += int(tchunks[ti]) * DT
            pos += n
        cores.append(dict(
            esrc_w=_wrap16(esrc),
            dstloc=np.ascontiguousarray(dlocf.reshape(nchunk, DT).T),
            dstlocT=np.ascontiguousarray(dlocf.reshape(nchunk, DT)),
            eattrT=np.ascontiguousarray(eat.T),
            hxidx_w=_wrap16(np.arange(k * NS, k * NS + NT * DT)),
        ))
    return cores, tchunks, epad


def _prep_pool(batch):
    """Per-core pooling matrix over the core's contiguous graph-id window,
    with 1/count baked in. Host overlap-adds windows afterwards (pure unshard
    glue; disjoint for the reference's uniform batch)."""
    batch = np.asarray(batch, np.int64)
    cnt = np.bincount(batch, minlength=G).astype(np.float32)
    g_lo = [int(batch[k * NS:(k + 1) * NS].min()) for k in range(NCORES)]
    g_hi = [int(batch[k * NS:(k + 1) * NS].max()) for k in range(NCORES)]
    ngk = min(max(max(h - l + 1 for l, h in zip(g_lo, g_hi)), 1), 128)
    pmats = []
    for k in range(NCORES):
        Pm = np.zeros((NT * DT, ngk), np.float32)
        bk = batch[k * NS:(k + 1) * NS]
        gl = np.clip(bk - g_lo[k], 0, ngk - 1)
        Pm[np.arange(NS), gl] = 1.0 / np.maximum(cnt[bk], 1.0)
        pm_dev = Pm.reshape(NT, DT, ngk).transpose(1, 0, 2).reshape(DT, NT * ngk)
        pmats.append(np.ascontiguousarray(pm_dev))
    return pmats, g_lo, ngk


# ==================== Bass program builders ====================

def _common_inputs(nc, mybir, epad, nchunk):
    f32, i16 = mybir.dt.float32, mybir.dt.int16
    t = {}
    t["esrc_w"] = nc.dram_tensor("esrc_w", [128, epad // 16], i16, kind="ExternalInput")
    t["hxidx_w"] = nc.dram_tensor("hxidx_w", [128, NT * DT // 16], i16, kind="ExternalInput")
    t["dstloc"] = nc.dram_tensor("dstloc", [128, nchunk], f32, kind="ExternalInput")
    t["dstlocT"] = nc.dram_tensor("dstlocT", [nchunk, 128], f32, kind="ExternalInput")
    t["iotap"] = nc.dram_tensor("iotap", [128, 1], f32, kind="ExternalInput")
    t["eattrT"] = nc.dram_tensor("eattrT", [4, epad], f32, kind="ExternalInput")
    t["iota8"] = nc.dram_tensor("iota8", [128, BS * 128], f32, kind="ExternalInput")
    t["we"] = nc.dram_tensor("we", [4, H], f32, kind="ExternalInput")
    t["b"] = nc.dram_tensor("b", [HC], f32, kind="ExternalInput")
    return t


def _load_common_sbuf(nc, tc, ctx, mybir, t):
    f32, i16 = mybir.dt.float32, mybir.dt.int16
    cpool = ctx.enter_context(tc.tile_pool(name="const", bufs=1))
    sb = {}
    ew = t["esrc_w"].shape[1]
    sb["esrc"] = cpool.tile([128, ew], i16, tag="esrc", name="esrc_sb")
    nc.sync.dma_start(out=sb["esrc"][:], in_=t["esrc_w"].ap())
    sb["hxidx"] = cpool.tile([128, NT * DT // 16], i16, tag="hxidx", name="hxidx_sb")
    nc.sync.dma_start(out=sb["hxidx"][:], in_=t["hxidx_w"].ap())
    sb["iota"] = cpool.tile([128, BS, 128], f32, tag="iota", name="iota_sb")
    nc.sync.dma_start(out=sb["iota"][:],
                      in_=t["iota8"].ap().rearrange("p (c x) -> p c x", c=BS))
    sb["web"] = cpool.tile([4, H], f32, tag="web", name="web_sb")
    nc.sync.dma_start(out=sb["web"][:], in_=t["we"].ap())
    sb["iotap"] = cpool.tile([128, 1], f32, tag="iotap", name="iotap_sb")
    nc.sync.dma_start(out=sb["iotap"][:], in_=t["iotap"].ap())
    sb["b"] = cpool.tile([128, HC], f32, tag="bb", name="b_sb")
    nc.sync.dma_start(out=sb["b"][:], in_=t["b"].ap()[None, :].to_broadcast([128, HC]))
    return sb, cpool


def _logit_view(ap3, mybir):
    """[P, n, TBW] table-row tile -> [P, n, 16] f32 view of the logit block."""
    if BF16:
        return ap3[:, :, HC:HC + 32].bitcast(mybir.dt.float32)
    return ap3[:, :, HC:HC + 16]


def _edge_phase(nc, mybir, ctx, tc, cfg, tbl, sb, relu, pool_cfg=None):
    """Per-layer edge phase + per-tile epilogue. Stores normalized tiles to
    cfg['store'] (if set) and/or accumulates graph pooling (pool_cfg)."""
    tchunks = cfg["tchunks"]
    f32 = mybir.dt.float32
    td = mybir.dt.bfloat16 if BF16 else f32

    gpool = ctx.enter_context(tc.tile_pool(name="gpool", bufs=4))
    spool = ctx.enter_context(tc.tile_pool(name="spool", bufs=3))
    ps_feat_p = ctx.enter_context(tc.tile_pool(name="psf", bufs=2, space="PSUM"))
    ps_stat_p = ctx.enter_context(tc.tile_pool(name="pss", bufs=2, space="PSUM"))
    ps_ae_p = ctx.enter_context(tc.tile_pool(name="psa", bufs=2, space="PSUM"))
    if pool_cfg is not None:
        ps_pool_p = ctx.enter_context(tc.tile_pool(name="psp", bufs=1, space="PSUM"))
        pool_ps = ps_pool_p.tile([pool_cfg["ngk"], 2], f32, tag="poolps", name="pool_ps")

    tbl_ap = tbl.ap()

    chunk0 = 0
    for ti in range(NT):
        nch = int(tchunks[ti])
        ps_feat = ps_feat_p.tile([DT, HC], f32, tag="feat")
        ps_stat = ps_stat_p.tile([DT, 17], f32, tag="stat")
        # local table rows for this dst tile; the f32 a_d block doubles as the
        # rhs of the per-chunk a_d broadcast matmul
        hx = spool.tile([DT, 1, TBW], td, tag="hx")
        nc.gpsimd.dma_gather(hx[:], tbl_ap,
                             sb["hxidx"][:, ti * 8:(ti + 1) * 8],
                             DT, DT, TBW)
        hxf = _logit_view(hx[:], mybir)        # [DT, 1, 16] f32
        first_chunk = True
        done = 0
        while done < nch:
            bs = min(BS, nch - done)
            c0 = chunk0 + done
            nidx = bs * DT
            hg = gpool.tile([DT, bs, TBW], td, tag="hg")
            nc.gpsimd.dma_gather(hg[:], tbl_ap,
                                 sb["esrc"][:, c0 * 8: c0 * 8 + nidx // 16],
                                 nidx, nidx, TBW, queue_num=(c0 // BS) % QN)
            hgf = _logit_view(hg[:], mybir)    # [DT, bs, 16] f32
            eat = spool.tile([4, bs * DT], f32, tag="eat")
            nc.sync.dma_start(out=eat[:],
                              in_=cfg["eattrT"].ap()[:, c0 * DT:(c0 + bs) * DT])
            dl = spool.tile([DT, bs], f32, tag="dl")
            nc.sync.dma_start(out=dl[:], in_=cfg["dstloc"].ap()[:, c0:c0 + bs])
            # dst-locals replicated across partitions for the transposed
            # scatter matrix (the partition broadcast comes from the DMA)
            dlb = spool.tile([DT, bs, DT], f32, tag="dlb")
            nc.sync.dma_start(
                out=dlb[:],
                in_=cfg["dstlocT"].ap()[None, c0:c0 + bs, :].to_broadcast(
                    [DT, bs, DT]))
            ct = spool.tile([DT, bs, DT], f32, tag="ct")
            nc.vector.tensor_scalar(ct[:], dlb[:], sb["iotap"][:], None,
                                    mybir.AluOpType.is_equal)

            ps_ae = ps_ae_p.tile([DT, bs, 16], f32, tag="ae")
            for c in range(bs):
                nc.tensor.matmul(out=ps_ae[:, c, 0:H],
                                 lhsT=eat[:, c * DT:(c + 1) * DT],
                                 rhs=sb["web"][:], start=True, stop=True)
                # a_d[dst_e] broadcast: CT.T @ a_d_tile
                nc.tensor.matmul(out=ps_ae[:, c, H:2 * H],
                                 lhsT=ct[:, c, :],
                                 rhs=hxf[:, 0, 8:16],
                                 start=True, stop=True)
            s = spool.tile([DT, bs, H], f32, tag="s")
            nc.vector.tensor_tensor(out=s[:], in0=hgf[:, :, 0:H],
                                    in1=ps_ae[:, :, H:2 * H],
                                    op=mybir.AluOpType.add)
            nc.vector.tensor_tensor(out=s[:], in0=s[:], in1=ps_ae[:, :, 0:H],
                                    op=mybir.AluOpType.add)
            s2 = spool.tile([DT, bs, H], f32, tag="s2")
            nc.vector.tensor_scalar_mul(s2[:], s[:], NEG)
            nc.vector.tensor_tensor(out=s[:], in0=s[:], in1=s2[:],
                                    op=mybir.AluOpType.max)
            wae = spool.tile([DT, bs, 17], td, tag="wae")
            nc.scalar.activation(wae[:, :, 0:H], s[:],
                                 mybir.ActivationFunctionType.Exp)
            nc.vector.tensor_copy(out=wae[:, :, H:2 * H], in_=ps_ae[:, :, 0:H])
            nc.vector.memset(wae[:, :, 2 * H:2 * H + 1], 1.0)
            cm = spool.tile([DT, bs, DT], td, tag="cm")
            nc.vector.tensor_tensor(
                out=cm[:], in0=dl[:, :, None].to_broadcast([DT, bs, DT]),
                in1=sb["iota"][:, 0:bs, :], op=mybir.AluOpType.is_equal)
            hg4 = hg[:, :, 0:HC].rearrange("p c (h x) -> p c h x", h=H)
            nc.vector.tensor_tensor(
                out=hg4, in0=hg4,
                in1=wae[:, :, 0:H].to_broadcast([DT, bs, H, C]),
                op=mybir.AluOpType.mult)
            for c in range(bs):
                last = (done + c == nch - 1)
                nc.tensor.matmul(out=ps_feat[:], lhsT=cm[:, c, :],
                                 rhs=hg[:, c, 0:HC],
                                 start=first_chunk, stop=last,
                                 skip_group_check=True)
                nc.tensor.matmul(out=ps_stat[:], lhsT=cm[:, c, :],
                                 rhs=wae[:, c, :],
                                 start=first_chunk, stop=last,
                                 skip_group_check=True)
                first_chunk = False
            done += bs
        chunk0 += nch

        # ---- per-tile epilogue ----
        dmax = spool.tile([DT, 1], f32, tag="dmax")
        nc.vector.tensor_scalar_max(dmax[:], ps_stat[:, 16:17], 1.0)
        rdeg = spool.tile([DT, 1], f32, tag="rdeg")
        nc.vector.reciprocal(rdeg[:], dmax[:])
        sl_ = spool.tile([DT, H], f32, tag="sl")
        nc.vector.tensor_tensor(out=sl_[:], in0=ps_stat[:, 8:16],
                                in1=rdeg[:].to_broadcast([DT, H]),
                                op=mybir.AluOpType.mult)
        nc.vector.tensor_tensor(out=sl_[:], in0=sl_[:], in1=hxf[:, 0, 0:8],
                                op=mybir.AluOpType.add)
        nc.vector.tensor_tensor(out=sl_[:], in0=sl_[:], in1=hxf[:, 0, 8:16],
                                op=mybir.AluOpType.add)
        sl2 = spool.tile([DT, H], f32, tag="sl2")
        nc.vector.tensor_scalar_mul(sl2[:], sl_[:], NEG)
        nc.vector.tensor_tensor(out=sl_[:], in0=sl_[:], in1=sl2[:],
                                op=mybir.AluOpType.max)
        wl = spool.tile([DT, H], f32, tag="wl")
        nc.scalar.activation(wl[:], sl_[:], mybir.ActivationFunctionType.Exp)
        den = spool.tile([DT, H], f32, tag="den")
        nc.vector.tensor_tensor(out=den[:], in0=ps_stat[:, 0:8], in1=wl[:],
                                op=mybir.AluOpType.add)
        rden = spool.tile([DT, H], f32, tag="rden")
        nc.vector.reciprocal(rden[:], den[:])
        out_t = spool.tile([DT, HC], f32, tag="outt")
        o4 = out_t[:].rearrange("p (h x) -> p h x", h=H)
        hx4 = hx[:, 0, 0:HC].rearrange("p (h x) -> p h x", h=H)
        for h in range(H):
            nc.scalar.activation(o4[:, h, :], hx4[:, h, :],
                                 mybir.ActivationFunctionType.Copy,
                                 scale=wl[:, h:h + 1])
        nc.vector.tensor_tensor(out=out_t[:], in0=out_t[:], in1=ps_feat[:],
                                op=mybir.AluOpType.add)
        for h in range(H):
            nc.scalar.activation(o4[:, h, :], o4[:, h, :],
                                 mybir.ActivationFunctionType.Copy,
                                 scale=rden[:, h:h + 1])
        nc.vector.tensor_tensor(out=out_t[:], in0=out_t[:], in1=sb["b"][:],
                                op=mybir.AluOpType.add)
        if relu:
            nc.vector.tensor_scalar_max(out_t[:], out_t[:], 0.0)
        if cfg.get("store") is not None:
            nc.sync.dma_start(out=cfg["store"].ap()[ti * DT:(ti + 1) * DT, :],
                              in_=out_t[:])
        if pool_cfg is not None:
            ngk = pool_cfg["ngk"]
            ytile = spool.tile([DT, 2], f32, tag="yt")
            tmp = spool.tile([DT, HC], f32, tag="ytmp")
            for j in range(2):
                nc.vector.tensor_tensor(out=tmp[:], in0=out_t[:],
                                        in1=sb["linw%d" % j][:],
                                        op=mybir.AluOpType.mult)
                nc.vector.tensor_reduce(out=ytile[:, j:j + 1], in_=tmp[:],
                                        axis=mybir.AxisListType.X,
                                        op=mybir.AluOpType.add)
            nc.tensor.matmul(out=pool_ps[:],
                             lhsT=sb["pmat"][:, ti * ngk:(ti + 1) * ngk],
                             rhs=ytile[:], start=(ti == 0), stop=(ti == NT - 1),
                             skip_group_check=True)

    if pool_cfg is not None:
        ysb = spool.tile([pool_cfg["ngk"], 2], f32, tag="ysb")
        nc.vector.tensor_tensor(out=ysb[:], in0=pool_ps[:], in1=sb["linb"][:],
                                op=mybir.AluOpType.add)
        nc.sync.dma_start(out=pool_cfg["yout"].ap()[:, :], in_=ysb[:])


def _store_table_rows(nc, mybir, dst_tile, c, psF, psL):
    """Write one node chunk into a [128, nb, TBW] table tile: features
    (cast to the table dtype) + the 16 f32 logits kept bit-exact."""
    f32 = mybir.dt.float32
    nc.vector.tensor_copy(out=dst_tile[:, c, 0:HC], in_=psF[:])
    if BF16:
        nc.vector.tensor_copy(
            out=dst_tile[:, c, HC:HC + 32].bitcast(f32), in_=psL[:])
    else:
        nc.vector.tensor_copy(out=dst_tile[:, c, HC:HC + 16], in_=psL[:])


def _build_launch1(meta):
    import concourse.bacc as bacc
    import concourse.tile as tile
    from concourse import mybir
    from contextlib import ExitStack
    from concourse.masks import make_identity

    f32 = mybir.dt.float32
    td = mybir.dt.bfloat16 if BF16 else f32
    epad, nchunk, tchunks = meta["epad"], meta["nchunk"], meta["tchunks"]

    nc = bacc.Bacc("TRN2", target_bir_lowering=False, debug=False)
    t = _common_inputs(nc, mybir, epad, nchunk)
    t["xtc"] = nc.dram_tensor("xtc", [NCH_A, 128, 128], f32, kind="ExternalInput")
    t["W1ext"] = nc.dram_tensor("W1ext", [128, HC + 16], f32, kind="ExternalInput")
    t["W2ext"] = nc.dram_tensor("W2ext", [512, HC + 16], f32, kind="ExternalInput")
    tbl = nc.dram_tensor("h1ext", [NPAD, TBW], td)
    x1out = nc.dram_tensor("x1out", [NT * DT, HC], f32)
    h2slice = nc.dram_tensor("h2slice", [NT * DT, TBW], td, kind="ExternalOutput")
    parts = os.environ.get("KGAT_L1_PARTS", "AEC")

    with tile.TileContext(nc) as tc, ExitStack() as ctx:
        sb, cpool = _load_common_sbuf(nc, tc, ctx, mybir, t)
        w1sb = cpool.tile([128, HC + 16], f32, tag="w1sb")
        nc.sync.dma_start(out=w1sb[:], in_=t["W1ext"].ap())
        w2sb = []
        for kc in range(4):
            w = cpool.tile([128, HC + 16], f32, tag="w2sb%d" % kc, name="w2sb%d" % kc)
            nc.sync.dma_start(out=w[:], in_=t["W2ext"].ap()[kc * 128:(kc + 1) * 128, :])
            w2sb.append(w)
        ident = cpool.tile([128, 128], f32, tag="ident")
        make_identity(nc, ident[:])

        # ---------------- phase A: h1ext = xpad @ W1ext ----------------
        with tc.tile_pool(name="pA", bufs=3) as pA, \
             tc.tile_pool(name="psA", bufs=2, space="PSUM") as psA_p, \
             tc.tile_pool(name="psA2", bufs=2, space="PSUM") as psA2_p:
            AB = 8
            for t0 in range(0, NCH_A, AB):
                nb = min(AB, NCH_A - t0)
                xb = pA.tile([128, nb, 128], f32, tag="xb")
                nc.sync.dma_start(
                    out=xb[:],
                    in_=t["xtc"].ap()[t0:t0 + nb].rearrange("c f n -> f c n"))
                hb = pA.tile([128, nb, TBW], td, tag="hb")
                if BF16:
                    nc.vector.memset(hb[:, :, HC + 32:TBW], 0.0)
                else:
                    nc.vector.memset(hb[:, :, HC + 16:TBW], 0.0)
                for c in range(nb):
                    psA = psA_p.tile([128, HC], f32, tag="psA")
                    psA2 = psA2_p.tile([128, 16], f32, tag="psA2")
                    nc.tensor.matmul(out=psA[:], lhsT=xb[:, c, :],
                                     rhs=w1sb[:, 0:HC], start=True, stop=True)
                    nc.tensor.matmul(out=psA2[:], lhsT=xb[:, c, :],
                                     rhs=w1sb[:, HC:HC + 16], start=True, stop=True)
                    _store_table_rows(nc, mybir, hb, c, psA, psA2)
                nc.sync.dma_start(
                    out=tbl.ap()[t0 * 128:(t0 + nb) * 128, :].rearrange(
                        "(c p) w -> p c w", p=128),
                    in_=hb[:])
        tc.strict_bb_all_engine_barrier()

        # ---------------- layer-1 edge phase ----------------
        if "E" in parts:
            cfg = dict(tchunks=tchunks, eattrT=t["eattrT"], dstloc=t["dstloc"],
                       dstlocT=t["dstlocT"], store=x1out)
            with ExitStack() as ectx:
                _edge_phase(nc, mybir, ectx, tc, cfg, tbl, sb, relu=True)
            tc.strict_bb_all_engine_barrier()

        # ---------------- phase C: h2slice = x1out @ W2ext ----------------
        if "C" in parts:
            with tc.tile_pool(name="pC", bufs=2) as pC, \
                 tc.tile_pool(name="psTr", bufs=2, space="PSUM") as psTr_p, \
                 tc.tile_pool(name="psH", bufs=2, space="PSUM") as psH_p, \
                 tc.tile_pool(name="psH2", bufs=2, space="PSUM") as psH2_p:
                for ti in range(NT):
                    x2 = pC.tile([128, HC], f32, tag="x2")
                    nc.sync.dma_start(out=x2[:],
                                      in_=x1out.ap()[ti * DT:(ti + 1) * DT, :])
                    x2t = pC.tile([128, HC], f32, tag="x2t")
                    for kc in range(4):
                        ptr = psTr_p.tile([128, 128], f32, tag="ptr")
                        nc.tensor.transpose(out=ptr[:],
                                            in_=x2[:, kc * 128:(kc + 1) * 128],
                                            identity=ident[:])
                        nc.vector.tensor_copy(out=x2t[:, kc * 128:(kc + 1) * 128],
                                              in_=ptr[:])
                    psH = psH_p.tile([128, HC], f32, tag="psH")
                    psH2 = psH2_p.tile([128, 16], f32, tag="psH2")
                    for kc in range(4):
                        nc.tensor.matmul(out=psH[:],
                                         lhsT=x2t[:, kc * 128:(kc + 1) * 128],
                                         rhs=w2sb[kc][:, 0:HC],
                                         start=(kc == 0), stop=(kc == 3))
                        nc.tensor.matmul(out=psH2[:],
                                         lhsT=x2t[:, kc * 128:(kc + 1) * 128],
                                         rhs=w2sb[kc][:, HC:HC + 16],
                                         start=(kc == 0), stop=(kc == 3))
                    h2t = pC.tile([128, 1, TBW], td, tag="h2t")
                    if BF16:
                        nc.vector.memset(h2t[:, :, HC + 32:TBW], 0.0)
                    else:
                        nc.vector.memset(h2t[:, :, HC + 16:TBW], 0.0)
                    _store_table_rows(nc, mybir, h2t, 0, psH, psH2)
                    nc.sync.dma_start(out=h2slice.ap()[ti * DT:(ti + 1) * DT, :],
                                      in_=h2t[:, 0, :])

    nc.compile()
    return nc


def _build_launch2(meta):
    import concourse.bacc as bacc
    import concourse.tile as tile
    from concourse import mybir
    from contextlib import ExitStack

    f32 = mybir.dt.float32
    td = mybir.dt.bfloat16 if BF16 else f32
    epad, nchunk, tchunks = meta["epad"], meta["nchunk"], meta["tchunks"]
    ngk = meta["ngk"]

    nc = bacc.Bacc("TRN2", target_bir_lowering=False, debug=False)
    t = _common_inputs(nc, mybir, epad, nchunk)
    t["h2ext"] = nc.dram_tensor("h2ext", [NPAD, TBW], td, kind="ExternalInput")
    t["pmat"] = nc.dram_tensor("pmat", [128, NT * ngk], f32, kind="ExternalInput")
    t["lin_w"] = nc.dram_tensor("lin_w", [HC, 2], f32, kind="ExternalInput")
    t["lin_b"] = nc.dram_tensor("lin_b", [2], f32, kind="ExternalInput")
    yout = nc.dram_tensor("yout", [ngk, 2], f32, kind="ExternalOutput")

    with tile.TileContext(nc) as tc, ExitStack() as ctx:
        sb, cpool = _load_common_sbuf(nc, tc, ctx, mybir, t)
        sb["pmat"] = cpool.tile([128, NT * ngk], f32, tag="pm", name="pm_sb")
        nc.sync.dma_start(out=sb["pmat"][:], in_=t["pmat"].ap())
        for j in range(2):
            lw = cpool.tile([128, HC], f32, tag="linw%d" % j, name="linw%d_sb" % j)
            nc.sync.dma_start(
                out=lw[:],
                in_=t["lin_w"].ap()[:, j:j + 1].rearrange("a b -> b a").to_broadcast([128, HC]))
            sb["linw%d" % j] = lw
        sb["linb"] = cpool.tile([ngk, 2], f32, tag="linb", name="linb_sb")
        nc.sync.dma_start(out=sb["linb"][:],
                          in_=t["lin_b"].ap()[None, :].to_broadcast([ngk, 2]))

        cfg = dict(tchunks=tchunks, eattrT=t["eattrT"], dstloc=t["dstloc"],
                   dstlocT=t["dstlocT"], store=None)
        pool_cfg = dict(ngk=ngk, yout=yout)
        with ExitStack() as ectx:
            _edge_phase(nc, mybir, ectx, tc, cfg, t["h2ext"], sb, relu=False,
                        pool_cfg=pool_cfg)

    nc.compile()
    return nc


# ==================== driver ====================

def _install_trace_shim():
    """Dev-only (KGAT_TRACE=1): register the axon NTFF profile hook that this
    image's antenv lacks, and keep profile artifacts local."""
    import sys, types
    try:
        from antenv import axon_hooks  # noqa: F401
        return
    except ImportError:
        pass
    try:
        from trn_agent_boot.trn_boot import _ntff_profile_via_ctypes
        mod = types.ModuleType("antenv.axon_hooks")
        mod._h = _ntff_profile_via_ctypes("/opt/axon/libaxon_pjrt.so")
        mod.set_axon_ntff_profile_hook = lambda h: setattr(mod, "_h", h)
        mod.get_axon_ntff_profile_hook = lambda: mod._h
        sys.modules["antenv.axon_hooks"] = mod
        import antenv
        antenv.axon_hooks = mod
        import concourse.bass_utils as bu
        bu.upload_artifacts = lambda d: d
    except Exception as e:  # pragma: no cover
        print(f"trace shim failed ({e}); falling back to untraced run")


def _run(nc, in_maps, sim_cores=None):
    global LAST_EXEC_TIMES
    if sim_cores is not None:
        from concourse.bass_interp import CoreSim
        out_names = [a.memorylocations[0].name
                     for a in nc.m.functions[0].allocations
                     if getattr(a, "kind", None) == "ExternalOutput"]
        outs = [None] * len(in_maps)
        for ci in sim_cores:
            # f32 logit bytes inside bf16 table rows false-positive the
            # simulator's NaN scan; disable it (bit-exact execution unchanged)
            s = CoreSim(nc, trace=False, require_finite=False, require_nnan=False)
            for k, v in in_maps[ci].items():
                s.tensor(k)[:] = v
            s.simulate(check_with_hw=False)
            outs[ci] = {n: np.array(s.tensor(n)) for n in out_names}
        return outs
    trace = os.environ.get("KGAT_TRACE", "0") == "1"
    if trace:
        _install_trace_shim()
    from concourse.bass_utils import run_bass_kernel_spmd
    import time as _time
    t0 = _time.time()
    res = run_bass_kernel_spmd(nc, in_maps, list(range(NCORES)), trace=trace)
    if os.environ.get("KGAT_VERBOSE", "0") == "1":
        print(f"launch wall {_time.time() - t0:.2f}s exec_ns={res.exec_time_ns}")
    if res.exec_time_ns is not None:
        LAST_EXEC_TIMES.append(res.exec_time_ns)
    return res.results


def _get_program(which, meta):
    key = (which, BF16, meta["epad"], tuple(meta["tchunks"]), meta.get("ngk"))
    if key not in _PROGRAMS:
        _PROGRAMS[key] = (_build_launch1 if which == 1 else _build_launch2)(meta)
    return _PROGRAMS[key]


def kernel(**inputs):
    global LAST_EXEC_TIMES
    LAST_EXEC_TIMES = []
    sim = os.environ.get("KGAT_SIM", "0") == "1"
    sim_cores = list(range(NCORES)) if sim else None

    x = np.asarray(inputs["x"], np.float32)
    ei = np.asarray(inputs["edge_index"], np.int64)
    batch = np.asarray(inputs["batch"], np.int64)
    edge_attr = np.asarray(inputs["edge_attr"], np.float32)

    W1ext, we1 = _prep_params(inputs["W1"], inputs["att_src1"], inputs["att_dst1"],
                              inputs["att_edge1"], inputs["We1"])
    W2ext, we2 = _prep_params(inputs["W2"], inputs["att_src2"], inputs["att_dst2"],
                              inputs["att_edge2"], inputs["We2"])
    b1 = np.asarray(inputs["b1"], np.float32)
    b2 = np.asarray(inputs["b2"], np.float32)
    lin_w = np.asarray(inputs["lin_w"], np.float32)
    lin_b = np.asarray(inputs["lin_b"], np.float32)

    cores, tchunks, epad = _prep_edges(ei[0], ei[1], edge_attr)
    pmats, g_lo, ngk = _prep_pool(batch)
    nchunk = epad // DT

    xpad = np.zeros((NPAD, F), np.float32)
    xpad[:N] = x
    xtc = np.ascontiguousarray(xpad.reshape(NCH_A, 128, F).transpose(0, 2, 1))
    iota8 = np.ascontiguousarray(
        np.tile(np.arange(128, dtype=np.float32), (128, BS)))
    iotap = np.arange(128, dtype=np.float32).reshape(128, 1).copy()

    meta = dict(epad=epad, nchunk=nchunk, tchunks=tchunks, ngk=ngk)

    def common_maps(k):
        ck = cores[k]
        return dict(esrc_w=ck["esrc_w"], hxidx_w=ck["hxidx_w"],
                    dstloc=ck["dstloc"], dstlocT=ck["dstlocT"],
                    eattrT=ck["eattrT"], iota8=iota8, iotap=iotap)

    nc1 = _get_program(1, meta)
    in_maps1 = []
    for k in range(NCORES):
        m = common_maps(k)
        m.update(xtc=xtc, W1ext=W1ext, W2ext=W2ext, we=we1, b=b1)
        in_maps1.append(m)
    res1 = _run(nc1, in_maps1, sim_cores=sim_cores)

    h2full = np.concatenate([res1[k]["h2slice"][:NS] for k in range(NCORES)], 0)
    h2pad = np.zeros((NPAD, TBW), h2full.dtype)
    h2pad[:N] = h2full

    nc2 = _get_program(2, meta)
    in_maps2 = []
    for k in range(NCORES):
        m = common_maps(k)
        m.update(h2ext=h2pad, pmat=pmats[k], we=we2, b=b2,
                 lin_w=lin_w, lin_b=lin_b)
        in_maps2.append(m)
    res2 = _run(nc2, in_maps2, sim_cores=sim_cores)

    y = np.zeros((G, 2), np.float32)
    for k in range(NCORES):
        yk = np.asarray(res2[k]["yout"], np.float32)
        for gi in range(ngk):
            g = g_lo[k] + gi
            if g < G:
                y[g] += yk[gi]
    return (y + lin_b[None, :]).astype(np.float32)


# revision 17
# speedup vs baseline: 1.4463x; 1.0096x over previous
"""Trainium2 Bass kernel for a 2-layer GAT (PyG GATConv, concat heads) +
global mean pool + linear head, distributed over 8 NeuronCores.

Strategy (self-contained; shapes hardcoded for this problem):
  - Destination-shard nodes across the 8 cores (2500 each); a core owns the
    edges whose destination lies in its slice. Graph pooling is shard-local.
  - he = edge_attr @ We is never materialized: its only use is
    a_e = he . att_e, which collapses to a_e = edge_attr @ we with the [4, 8]
    matrix we[d, h] = We[d, h*64:(h+1)*64] @ att_e[h] (host weight folding).
  - Self-loops (fill_value='mean') are folded analytically: the self-loop's
    a_e equals segment_sum(a_e)/max(deg, 1); both stats are accumulated as
    extra columns of the edge-aggregation matmul.
  - Softmax without max-subtraction: out_i = sum_e exp(r_e) h_src / sum exp(r_e)
    is mathematically identical to the max-normalized form (logits are O(10)).
  - Edge aggregation per 128-destination tile = PE matmul with the one-hot
    scatter matrix C[e, d] = (dst_local[e] == d), built on DVE from an iota
    constant; messages are dma_gather'ed source rows scaled by exp-logits.
    a_d[dst_e] is broadcast to edges with a second matmul using C^T (built
    from a partition-replicated dst-local row), avoiding a per-edge gather.
  - Node-table rows store features in bf16 (KGAT_BF16=1, default) with the
    16 attention logits (a_s|a_d) kept exact as f32 bytes inside the row;
    rows are 1280 B, satisfying dma_gather's 256 B-multiple constraint.
  - Two SPMD launches: launch 1 builds the layer-1 node table x @ W1ext,
    runs the layer-1 edge phase, and emits each core's slice of the layer-2
    table h2 = relu(out1) @ W2ext. The host concatenates slices (pure data
    movement) and launch 2 runs the layer-2 edge phase + pooling + head.

Graded entry point: kernel(**inputs) -> np.ndarray [200, 2] float32.
"""

import os
import numpy as np

# -------------------- problem constants --------------------
N, F, H, C, HC, ED, E, G = 20000, 128, 8, 64, 512, 4, 320000, 200
NEG = 0.2
NCORES = 8
NS = N // NCORES            # 2500 destination nodes per core
DT = 128                    # destinations per tile (PSUM partition dim)
NT = (NS + DT - 1) // DT    # 20 dst tiles per core
NPAD = ((N + 127) // 128) * 128   # 20096 table rows (pad rows zero)
NCH_A = NPAD // 128         # 157 node chunks in phase A
BS = 8                      # max 128-edge chunks per gather batch
QN = int(os.environ.get("KGAT_QN", "1"))   # SWDGE queues to rotate gathers over
# (dma_gather with num_idxs=2048 crashes the exec unit; 1024 is solid)

BF16 = os.environ.get("KGAT_BF16", "1") == "1"
# table row: [512 feat | 16 f32 logits (a_s|a_d) | pad]; bf16 rows are 640
# elems = 1280 B, f32 rows 576 elems = 2304 B (both % 256 B as required).
TBW = 640 if BF16 else 576

LAST_EXEC_TIMES = []        # ns per launch (for the test harness)
_PROGRAMS = {}


def _wrap16(a):
    """dma_gather index layout: idx j sits at [j % 16, j // 16] (int16),
    replicated across the 8 groups of 16 partitions."""
    a = np.asarray(a, np.int64)
    assert a.size % 16 == 0
    w = a.astype(np.int16).reshape(a.size // 16, 16).T
    return np.ascontiguousarray(np.tile(w, (8, 1)))


# ==================== host-side preprocessing ====================

def _prep_params(W, att_s, att_d, att_e, We):
    W = np.asarray(W, np.float32)
    att_s = np.asarray(att_s, np.float32)
    att_d = np.asarray(att_d, np.float32)
    att_e = np.asarray(att_e, np.float32)
    We = np.asarray(We, np.float32)
    As = np.zeros((HC, H), np.float32)
    Ad = np.zeros((HC, H), np.float32)
    for h in range(H):
        As[h * C:(h + 1) * C, h] = att_s[h]
        Ad[h * C:(h + 1) * C, h] = att_d[h]
    Wext = np.zeros((W.shape[0], HC + 16), np.float32)
    Wext[:, :HC] = W
    Wext[:, HC:HC + H] = W @ As
    Wext[:, HC + H:HC + 2 * H] = W @ Ad
    we = np.zeros((ED, H), np.float32)
    for h in range(H):
        we[:, h] = We[:, h * C:(h + 1) * C] @ att_e[h]
    return Wext, we


def _prep_edges(src, dst, edge_attr):
    counts = np.zeros((NCORES, NT), np.int64)
    per_core = []
    for k in range(NCORES):
        m = (dst >= k * NS) & (dst < (k + 1) * NS)
        s, d, ea = src[m], dst[m], edge_attr[m]
        dloc = d - k * NS
        t = dloc // DT
        order = np.argsort(t, kind="stable")
        s, dloc, ea = s[order], dloc[order], ea[order]
        counts[k] = np.bincount(t[order], minlength=NT)
        per_core.append((s, dloc, ea))
    tchunks = np.maximum(1, (counts.max(axis=0) + DT - 1) // DT).astype(np.int64)
    epad = int(tchunks.sum()) * DT
    nchunk = epad // DT
    cores = []
    for k in range(NCORES):
        s, dloc, ea = per_core[k]
        esrc = np.zeros(epad, np.int64)
        dlocf = np.full(epad, 999.0, np.float32)
        eat = np.zeros((epad, ED), np.float32)
        off = pos = 0
        for ti in range(NT):
            n = int(counts[k, ti])
            sl = slice(pos, pos + n)
            esrc[off:off + n] = s[sl]
            dlocf[off:off + n] = (dloc[sl] - ti * DT).astype(np.float32)
            eat[off:off + n] = ea[sl]
            off += int(tchunks[ti]) * DT
            pos += n
        cores.append(dict(
            esrc_w=_wrap16(esrc),
            dstloc=np.ascontiguousarray(dlocf.reshape(nchunk, DT).T),
            dstlocT=np.ascontiguousarray(dlocf.reshape(nchunk, DT)),
            eattrT=np.ascontiguousarray(eat.T),
            hxidx_w=_wrap16(np.arange(k * NS, k * NS + NT * DT)),
        ))
    return cores, tchunks, epad


def _prep_pool(batch):
    """Per-core pooling matrix over the core's contiguous graph-id window,
    with 1/count baked in. Host overlap-adds windows afterwards (pure unshard
    glue; disjoint for the reference's uniform batch)."""
    batch = np.asarray(batch, np.int64)
    cnt = np.bincount(batch, minlength=G).astype(np.float32)
    g_lo = [int(batch[k * NS:(k + 1) * NS].min()) for k in range(NCORES)]
    g_hi = [int(batch[k * NS:(k + 1) * NS].max()) for k in range(NCORES)]
    ngk = min(max(max(h - l + 1 for l, h in zip(g_lo, g_hi)), 1), 128)
    pmats = []
    for k in range(NCORES):
        Pm = np.zeros((NT * DT, ngk), np.float32)
        bk = batch[k * NS:(k + 1) * NS]
        gl = np.clip(bk - g_lo[k], 0, ngk - 1)
        Pm[np.arange(NS), gl] = 1.0 / np.maximum(cnt[bk], 1.0)
        pm_dev = Pm.reshape(NT, DT, ngk).transpose(1, 0, 2).reshape(DT, NT * ngk)
        pmats.append(np.ascontiguousarray(pm_dev))
    return pmats, g_lo, ngk


# ==================== Bass program builders ====================

def _common_inputs(nc, mybir, epad, nchunk):
    f32, i16 = mybir.dt.float32, mybir.dt.int16
    t = {}
    t["esrc_w"] = nc.dram_tensor("esrc_w", [128, epad // 16], i16, kind="ExternalInput")
    t["hxidx_w"] = nc.dram_tensor("hxidx_w", [128, NT * DT // 16], i16, kind="ExternalInput")
    t["dstloc"] = nc.dram_tensor("dstloc", [128, nchunk], f32, kind="ExternalInput")
    t["dstlocT"] = nc.dram_tensor("dstlocT", [nchunk, 128], f32, kind="ExternalInput")
    t["iotap"] = nc.dram_tensor("iotap", [128, 1], f32, kind="ExternalInput")
    t["eattrT"] = nc.dram_tensor("eattrT", [4, epad], f32, kind="ExternalInput")
    t["iota8"] = nc.dram_tensor("iota8", [128, BS * 128], f32, kind="ExternalInput")
    t["we"] = nc.dram_tensor("we", [4, H], f32, kind="ExternalInput")
    t["b"] = nc.dram_tensor("b", [HC], f32, kind="ExternalInput")
    return t


def _load_common_sbuf(nc, tc, ctx, mybir, t):
    f32, i16 = mybir.dt.float32, mybir.dt.int16
    cpool = ctx.enter_context(tc.tile_pool(name="const", bufs=1))
    sb = {}
    ew = t["esrc_w"].shape[1]
    sb["esrc"] = cpool.tile([128, ew], i16, tag="esrc", name="esrc_sb")
    nc.sync.dma_start(out=sb["esrc"][:], in_=t["esrc_w"].ap())
    sb["hxidx"] = cpool.tile([128, NT * DT // 16], i16, tag="hxidx", name="hxidx_sb")
    nc.sync.dma_start(out=sb["hxidx"][:], in_=t["hxidx_w"].ap())
    sb["iota"] = cpool.tile([128, BS, 128], f32, tag="iota", name="iota_sb")
    nc.sync.dma_start(out=sb["iota"][:],
                      in_=t["iota8"].ap().rearrange("p (c x) -> p c x", c=BS))
    sb["web"] = cpool.tile([4, H], f32, tag="web", name="web_sb")
    nc.sync.dma_start(out=sb["web"][:], in_=t["we"].ap())
    sb["iotap"] = cpool.tile([128, 1], f32, tag="iotap", name="iotap_sb")
    nc.sync.dma_start(out=sb["iotap"][:], in_=t["iotap"].ap())
    sb["b"] = cpool.tile([128, HC], f32, tag="bb", name="b_sb")
    nc.sync.dma_start(out=sb["b"][:], in_=t["b"].ap()[None, :].to_broadcast([128, HC]))
    return sb, cpool


def _logit_view(ap3, mybir):
    """[P, n, TBW] table-row tile -> [P, n, 16] f32 view of the logit block."""
    if BF16:
        return ap3[:, :, HC:HC + 32].bitcast(mybir.dt.float32)
    return ap3[:, :, HC:HC + 16]


def _edge_phase(nc, mybir, ctx, tc, cfg, tbl, sb, relu, pool_cfg=None):
    """Per-layer edge phase + per-tile epilogue. Stores normalized tiles to
    cfg['store'] (if set) and/or accumulates graph pooling (pool_cfg)."""
    tchunks = cfg["tchunks"]
    f32 = mybir.dt.float32
    td = mybir.dt.bfloat16 if BF16 else f32

    gpool = ctx.enter_context(tc.tile_pool(name="gpool", bufs=4))
    spool = ctx.enter_context(tc.tile_pool(name="spool", bufs=3))
    ps_feat_p = ctx.enter_context(tc.tile_pool(name="psf", bufs=2, space="PSUM"))
    ps_stat_p = ctx.enter_context(tc.tile_pool(name="pss", bufs=2, space="PSUM"))
    ps_ae_p = ctx.enter_context(tc.tile_pool(name="psa", bufs=2, space="PSUM"))
    if pool_cfg is not None:
        ps_pool_p = ctx.enter_context(tc.tile_pool(name="psp", bufs=1, space="PSUM"))
        pool_ps = ps_pool_p.tile([pool_cfg["ngk"], 2], f32, tag="poolps", name="pool_ps")

    tbl_ap = tbl.ap()

    chunk0 = 0
    for ti in range(NT):
        nch = int(tchunks[ti])
        ps_feat = ps_feat_p.tile([DT, HC], f32, tag="feat")
        ps_stat = ps_stat_p.tile([DT, 17], f32, tag="stat")
        # local table rows for this dst tile; the f32 a_d block doubles as the
        # rhs of the per-chunk a_d broadcast matmul
        hx = spool.tile([DT, 1, TBW], td, tag="hx")
        nc.gpsimd.dma_gather(hx[:], tbl_ap,
                             sb["hxidx"][:, ti * 8:(ti + 1) * 8],
                             DT, DT, TBW)
        hxf = _logit_view(hx[:], mybir)        # [DT, 1, 16] f32
        first_chunk = True
        done = 0
        while done < nch:
            bs = min(BS, nch - done)
            c0 = chunk0 + done
            nidx = bs * DT
            hg = gpool.tile([DT, bs, TBW], td, tag="hg")
            nc.gpsimd.dma_gather(hg[:], tbl_ap,
                                 sb["esrc"][:, c0 * 8: c0 * 8 + nidx // 16],
                                 nidx, nidx, TBW, queue_num=(c0 // BS) % QN,
                                 single_packet=os.environ.get("KGAT_SP", "1") == "1")
            hgf = _logit_view(hg[:], mybir)    # [DT, bs, 16] f32
            eat = spool.tile([4, bs * DT], f32, tag="eat")
            nc.sync.dma_start(out=eat[:],
                              in_=cfg["eattrT"].ap()[:, c0 * DT:(c0 + bs) * DT])
            dl = spool.tile([DT, bs], f32, tag="dl")
            nc.sync.dma_start(out=dl[:], in_=cfg["dstloc"].ap()[:, c0:c0 + bs])
            # dst-locals replicated across partitions for the transposed
            # scatter matrix (the partition broadcast comes from the DMA)
            dlb = spool.tile([DT, bs, DT], f32, tag="dlb")
            nc.sync.dma_start(
                out=dlb[:],
                in_=cfg["dstlocT"].ap()[None, c0:c0 + bs, :].to_broadcast(
                    [DT, bs, DT]))
            ct = spool.tile([DT, bs, DT], f32, tag="ct")
            nc.vector.tensor_scalar(ct[:], dlb[:], sb["iotap"][:], None,
                                    mybir.AluOpType.is_equal)

            ps_ae = ps_ae_p.tile([DT, bs, 16], f32, tag="ae")
            for c in range(bs):
                nc.tensor.matmul(out=ps_ae[:, c, 0:H],
                                 lhsT=eat[:, c * DT:(c + 1) * DT],
                                 rhs=sb["web"][:], start=True, stop=True)
                # a_d[dst_e] broadcast: CT.T @ a_d_tile
                nc.tensor.matmul(out=ps_ae[:, c, H:2 * H],
                                 lhsT=ct[:, c, :],
                                 rhs=hxf[:, 0, 8:16],
                                 start=True, stop=True)
            s = spool.tile([DT, bs, H], f32, tag="s")
            nc.vector.tensor_tensor(out=s[:], in0=hgf[:, :, 0:H],
                                    in1=ps_ae[:, :, H:2 * H],
                                    op=mybir.AluOpType.add)
            nc.vector.tensor_tensor(out=s[:], in0=s[:], in1=ps_ae[:, :, 0:H],
                                    op=mybir.AluOpType.add)
            s2 = spool.tile([DT, bs, H], f32, tag="s2")
            nc.vector.tensor_scalar_mul(s2[:], s[:], NEG)
            nc.vector.tensor_tensor(out=s[:], in0=s[:], in1=s2[:],
                                    op=mybir.AluOpType.max)
            wae = spool.tile([DT, bs, 17], td, tag="wae")
            nc.scalar.activation(wae[:, :, 0:H], s[:],
                                 mybir.ActivationFunctionType.Exp)
            nc.vector.tensor_copy(out=wae[:, :, H:2 * H], in_=ps_ae[:, :, 0:H])
            nc.vector.memset(wae[:, :, 2 * H:2 * H + 1], 1.0)
            cm = spool.tile([DT, bs, DT], td, tag="cm")
            nc.vector.tensor_tensor(
                out=cm[:], in0=dl[:, :, None].to_broadcast([DT, bs, DT]),
                in1=sb["iota"][:, 0:bs, :], op=mybir.AluOpType.is_equal)
            hg4 = hg[:, :, 0:HC].rearrange("p c (h x) -> p c h x", h=H)
            nc.vector.tensor_tensor(
                out=hg4, in0=hg4,
                in1=wae[:, :, 0:H].to_broadcast([DT, bs, H, C]),
                op=mybir.AluOpType.mult)
            for c in range(bs):
                last = (done + c == nch - 1)
                nc.tensor.matmul(out=ps_feat[:], lhsT=cm[:, c, :],
                                 rhs=hg[:, c, 0:HC],
                                 start=first_chunk, stop=last,
                                 skip_group_check=True)
                nc.tensor.matmul(out=ps_stat[:], lhsT=cm[:, c, :],
                                 rhs=wae[:, c, :],
                                 start=first_chunk, stop=last,
                                 skip_group_check=True)
                first_chunk = False
            done += bs
        chunk0 += nch

        # ---- per-tile epilogue ----
        dmax = spool.tile([DT, 1], f32, tag="dmax")
        nc.vector.tensor_scalar_max(dmax[:], ps_stat[:, 16:17], 1.0)
        rdeg = spool.tile([DT, 1], f32, tag="rdeg")
        nc.vector.reciprocal(rdeg[:], dmax[:])
        sl_ = spool.tile([DT, H], f32, tag="sl")
        nc.vector.tensor_tensor(out=sl_[:], in0=ps_stat[:, 8:16],
                                in1=rdeg[:].to_broadcast([DT, H]),
                                op=mybir.AluOpType.mult)
        nc.vector.tensor_tensor(out=sl_[:], in0=sl_[:], in1=hxf[:, 0, 0:8],
                                op=mybir.AluOpType.add)
        nc.vector.tensor_tensor(out=sl_[:], in0=sl_[:], in1=hxf[:, 0, 8:16],
                                op=mybir.AluOpType.add)
        sl2 = spool.tile([DT, H], f32, tag="sl2")
        nc.vector.tensor_scalar_mul(sl2[:], sl_[:], NEG)
        nc.vector.tensor_tensor(out=sl_[:], in0=sl_[:], in1=sl2[:],
                                op=mybir.AluOpType.max)
        wl = spool.tile([DT, H], f32, tag="wl")
        nc.scalar.activation(wl[:], sl_[:], mybir.ActivationFunctionType.Exp)
        den = spool.tile([DT, H], f32, tag="den")
        nc.vector.tensor_tensor(out=den[:], in0=ps_stat[:, 0:8], in1=wl[:],
                                op=mybir.AluOpType.add)
        rden = spool.tile([DT, H], f32, tag="rden")
        nc.vector.reciprocal(rden[:], den[:])
        out_t = spool.tile([DT, HC], f32, tag="outt")
        o4 = out_t[:].rearrange("p (h x) -> p h x", h=H)
        hx4 = hx[:, 0, 0:HC].rearrange("p (h x) -> p h x", h=H)
        for h in range(H):
            nc.scalar.activation(o4[:, h, :], hx4[:, h, :],
                                 mybir.ActivationFunctionType.Copy,
                                 scale=wl[:, h:h + 1])
        nc.vector.tensor_tensor(out=out_t[:], in0=out_t[:], in1=ps_feat[:],
                                op=mybir.AluOpType.add)
        for h in range(H):
            nc.scalar.activation(o4[:, h, :], o4[:, h, :],
                                 mybir.ActivationFunctionType.Copy,
                                 scale=rden[:, h:h + 1])
        nc.vector.tensor_tensor(out=out_t[:], in0=out_t[:], in1=sb["b"][:],
                                op=mybir.AluOpType.add)
        if relu:
            nc.vector.tensor_scalar_max(out_t[:], out_t[:], 0.0)
        if cfg.get("store") is not None:
            nc.sync.dma_start(out=cfg["store"].ap()[ti * DT:(ti + 1) * DT, :],
                              in_=out_t[:])
        if pool_cfg is not None:
            ngk = pool_cfg["ngk"]
            ytile = spool.tile([DT, 2], f32, tag="yt")
            tmp = spool.tile([DT, HC], f32, tag="ytmp")
            for j in range(2):
                nc.vector.tensor_tensor(out=tmp[:], in0=out_t[:],
                                        in1=sb["linw%d" % j][:],
                                        op=mybir.AluOpType.mult)
                nc.vector.tensor_reduce(out=ytile[:, j:j + 1], in_=tmp[:],
                                        axis=mybir.AxisListType.X,
                                        op=mybir.AluOpType.add)
            nc.tensor.matmul(out=pool_ps[:],
                             lhsT=sb["pmat"][:, ti * ngk:(ti + 1) * ngk],
                             rhs=ytile[:], start=(ti == 0), stop=(ti == NT - 1),
                             skip_group_check=True)

    if pool_cfg is not None:
        ysb = spool.tile([pool_cfg["ngk"], 2], f32, tag="ysb")
        nc.vector.tensor_tensor(out=ysb[:], in0=pool_ps[:], in1=sb["linb"][:],
                                op=mybir.AluOpType.add)
        nc.sync.dma_start(out=pool_cfg["yout"].ap()[:, :], in_=ysb[:])


def _store_table_rows(nc, mybir, dst_tile, c, psF, psL):
    """Write one node chunk into a [128, nb, TBW] table tile: features
    (cast to the table dtype) + the 16 f32 logits kept bit-exact."""
    f32 = mybir.dt.float32
    nc.vector.tensor_copy(out=dst_tile[:, c, 0:HC], in_=psF[:])
    if BF16:
        nc.vector.tensor_copy(
            out=dst_tile[:, c, HC:HC + 32].bitcast(f32), in_=psL[:])
    else:
        nc.vector.tensor_copy(out=dst_tile[:, c, HC:HC + 16], in_=psL[:])


def _build_launch1(meta):
    import concourse.bacc as bacc
    import concourse.tile as tile
    from concourse import mybir
    from contextlib import ExitStack
    from concourse.masks import make_identity

    f32 = mybir.dt.float32
    td = mybir.dt.bfloat16 if BF16 else f32
    epad, nchunk, tchunks = meta["epad"], meta["nchunk"], meta["tchunks"]

    nc = bacc.Bacc("TRN2", target_bir_lowering=False, debug=False)
    t = _common_inputs(nc, mybir, epad, nchunk)
    t["xtc"] = nc.dram_tensor("xtc", [NCH_A, 128, 128], f32, kind="ExternalInput")
    t["W1ext"] = nc.dram_tensor("W1ext", [128, HC + 16], f32, kind="ExternalInput")
    t["W2ext"] = nc.dram_tensor("W2ext", [512, HC + 16], f32, kind="ExternalInput")
    tbl = nc.dram_tensor("h1ext", [NPAD, TBW], td)
    x1out = nc.dram_tensor("x1out", [NT * DT, HC], f32)
    h2slice = nc.dram_tensor("h2slice", [NT * DT, TBW], td, kind="ExternalOutput")
    parts = os.environ.get("KGAT_L1_PARTS", "AEC")

    with tile.TileContext(nc) as tc, ExitStack() as ctx:
        sb, cpool = _load_common_sbuf(nc, tc, ctx, mybir, t)
        w1sb = cpool.tile([128, HC + 16], f32, tag="w1sb")
        nc.sync.dma_start(out=w1sb[:], in_=t["W1ext"].ap())
        w2sb = []
        for kc in range(4):
            w = cpool.tile([128, HC + 16], f32, tag="w2sb%d" % kc, name="w2sb%d" % kc)
            nc.sync.dma_start(out=w[:], in_=t["W2ext"].ap()[kc * 128:(kc + 1) * 128, :])
            w2sb.append(w)
        ident = cpool.tile([128, 128], f32, tag="ident")
        make_identity(nc, ident[:])

        # ---------------- phase A: h1ext = xpad @ W1ext ----------------
        with tc.tile_pool(name="pA", bufs=3) as pA, \
             tc.tile_pool(name="psA", bufs=2, space="PSUM") as psA_p, \
             tc.tile_pool(name="psA2", bufs=2, space="PSUM") as psA2_p:
            AB = 8
            for t0 in range(0, NCH_A, AB):
                nb = min(AB, NCH_A - t0)
                xb = pA.tile([128, nb, 128], f32, tag="xb")
                nc.sync.dma_start(
                    out=xb[:],
                    in_=t["xtc"].ap()[t0:t0 + nb].rearrange("c f n -> f c n"))
                hb = pA.tile([128, nb, TBW], td, tag="hb")
                if BF16:
                    nc.vector.memset(hb[:, :, HC + 32:TBW], 0.0)
                else:
                    nc.vector.memset(hb[:, :, HC + 16:TBW], 0.0)
                for c in range(nb):
                    psA = psA_p.tile([128, HC], f32, tag="psA")
                    psA2 = psA2_p.tile([128, 16], f32, tag="psA2")
                    nc.tensor.matmul(out=psA[:], lhsT=xb[:, c, :],
                                     rhs=w1sb[:, 0:HC], start=True, stop=True)
                    nc.tensor.matmul(out=psA2[:], lhsT=xb[:, c, :],
                                     rhs=w1sb[:, HC:HC + 16], start=True, stop=True)
                    _store_table_rows(nc, mybir, hb, c, psA, psA2)
                nc.sync.dma_start(
                    out=tbl.ap()[t0 * 128:(t0 + nb) * 128, :].rearrange(
                        "(c p) w -> p c w", p=128),
                    in_=hb[:])
        tc.strict_bb_all_engine_barrier()

        # ---------------- layer-1 edge phase ----------------
        if "E" in parts:
            cfg = dict(tchunks=tchunks, eattrT=t["eattrT"], dstloc=t["dstloc"],
                       dstlocT=t["dstlocT"], store=x1out)
            with ExitStack() as ectx:
                _edge_phase(nc, mybir, ectx, tc, cfg, tbl, sb, relu=True)
            tc.strict_bb_all_engine_barrier()

        # ---------------- phase C: h2slice = x1out @ W2ext ----------------
        if "C" in parts:
            with tc.tile_pool(name="pC", bufs=2) as pC, \
                 tc.tile_pool(name="psTr", bufs=2, space="PSUM") as psTr_p, \
                 tc.tile_pool(name="psH", bufs=2, space="PSUM") as psH_p, \
                 tc.tile_pool(name="psH2", bufs=2, space="PSUM") as psH2_p:
                for ti in range(NT):
                    x2 = pC.tile([128, HC], f32, tag="x2")
                    nc.sync.dma_start(out=x2[:],
                                      in_=x1out.ap()[ti * DT:(ti + 1) * DT, :])
                    x2t = pC.tile([128, HC], f32, tag="x2t")
                    for kc in range(4):
                        ptr = psTr_p.tile([128, 128], f32, tag="ptr")
                        nc.tensor.transpose(out=ptr[:],
                                            in_=x2[:, kc * 128:(kc + 1) * 128],
                                            identity=ident[:])
                        nc.vector.tensor_copy(out=x2t[:, kc * 128:(kc + 1) * 128],
                                              in_=ptr[:])
                    psH = psH_p.tile([128, HC], f32, tag="psH")
                    psH2 = psH2_p.tile([128, 16], f32, tag="psH2")
                    for kc in range(4):
                        nc.tensor.matmul(out=psH[:],
                                         lhsT=x2t[:, kc * 128:(kc + 1) * 128],
                                         rhs=w2sb[kc][:, 0:HC],
                                         start=(kc == 0), stop=(kc == 3))
                        nc.tensor.matmul(out=psH2[:],
                                         lhsT=x2t[:, kc * 128:(kc + 1) * 128],
                                         rhs=w2sb[kc][:, HC:HC + 16],
                                         start=(kc == 0), stop=(kc == 3))
                    h2t = pC.tile([128, 1, TBW], td, tag="h2t")
                    if BF16:
                        nc.vector.memset(h2t[:, :, HC + 32:TBW], 0.0)
                    else:
                        nc.vector.memset(h2t[:, :, HC + 16:TBW], 0.0)
                    _store_table_rows(nc, mybir, h2t, 0, psH, psH2)
                    nc.sync.dma_start(out=h2slice.ap()[ti * DT:(ti + 1) * DT, :],
                                      in_=h2t[:, 0, :])

    nc.compile()
    return nc


def _build_launch2(meta):
    import concourse.bacc as bacc
    import concourse.tile as tile
    from concourse import mybir
    from contextlib import ExitStack

    f32 = mybir.dt.float32
    td = mybir.dt.bfloat16 if BF16 else f32
    epad, nchunk, tchunks = meta["epad"], meta["nchunk"], meta["tchunks"]
    ngk = meta["ngk"]

    nc = bacc.Bacc("TRN2", target_bir_lowering=False, debug=False)
    t = _common_inputs(nc, mybir, epad, nchunk)
    t["h2ext"] = nc.dram_tensor("h2ext", [NPAD, TBW], td, kind="ExternalInput")
    t["pmat"] = nc.dram_tensor("pmat", [128, NT * ngk], f32, kind="ExternalInput")
    t["lin_w"] = nc.dram_tensor("lin_w", [HC, 2], f32, kind="ExternalInput")
    t["lin_b"] = nc.dram_tensor("lin_b", [2], f32, kind="ExternalInput")
    yout = nc.dram_tensor("yout", [ngk, 2], f32, kind="ExternalOutput")

    with tile.TileContext(nc) as tc, ExitStack() as ctx:
        sb, cpool = _load_common_sbuf(nc, tc, ctx, mybir, t)
        sb["pmat"] = cpool.tile([128, NT * ngk], f32, tag="pm", name="pm_sb")
        nc.sync.dma_start(out=sb["pmat"][:], in_=t["pmat"].ap())
        for j in range(2):
            lw = cpool.tile([128, HC], f32, tag="linw%d" % j, name="linw%d_sb" % j)
            nc.sync.dma_start(
                out=lw[:],
                in_=t["lin_w"].ap()[:, j:j + 1].rearrange("a b -> b a").to_broadcast([128, HC]))
            sb["linw%d" % j] = lw
        sb["linb"] = cpool.tile([ngk, 2], f32, tag="linb", name="linb_sb")
        nc.sync.dma_start(out=sb["linb"][:],
                          in_=t["lin_b"].ap()[None, :].to_broadcast([ngk, 2]))

        cfg = dict(tchunks=tchunks, eattrT=t["eattrT"], dstloc=t["dstloc"],
                   dstlocT=t["dstlocT"], store=None)
        pool_cfg = dict(ngk=ngk, yout=yout)
        with ExitStack() as ectx:
            _edge_phase(nc, mybir, ectx, tc, cfg, t["h2ext"], sb, relu=False,
                        pool_cfg=pool_cfg)

    nc.compile()
    return nc


# ==================== driver ====================

def _install_trace_shim():
    """Dev-only (KGAT_TRACE=1): register the axon NTFF profile hook that this
    image's antenv lacks, and keep profile artifacts local."""
    import sys, types
    try:
        from antenv import axon_hooks  # noqa: F401
        return
    except ImportError:
        pass
    try:
        from trn_agent_boot.trn_boot import _ntff_profile_via_ctypes
        mod = types.ModuleType("antenv.axon_hooks")
        mod._h = _ntff_profile_via_ctypes("/opt/axon/libaxon_pjrt.so")
        mod.set_axon_ntff_profile_hook = lambda h: setattr(mod, "_h", h)
        mod.get_axon_ntff_profile_hook = lambda: mod._h
        sys.modules["antenv.axon_hooks"] = mod
        import antenv
        antenv.axon_hooks = mod
        import concourse.bass_utils as bu
        bu.upload_artifacts = lambda d: d
    except Exception as e:  # pragma: no cover
        print(f"trace shim failed ({e}); falling back to untraced run")


def _run(nc, in_maps, sim_cores=None):
    global LAST_EXEC_TIMES
    if sim_cores is not None:
        from concourse.bass_interp import CoreSim
        out_names = [a.memorylocations[0].name
                     for a in nc.m.functions[0].allocations
                     if getattr(a, "kind", None) == "ExternalOutput"]
        outs = [None] * len(in_maps)
        for ci in sim_cores:
            # f32 logit bytes inside bf16 table rows false-positive the
            # simulator's NaN scan; disable it (bit-exact execution unchanged)
            s = CoreSim(nc, trace=False, require_finite=False, require_nnan=False)
            for k, v in in_maps[ci].items():
                s.tensor(k)[:] = v
            s.simulate(check_with_hw=False)
            outs[ci] = {n: np.array(s.tensor(n)) for n in out_names}
        return outs
    trace = os.environ.get("KGAT_TRACE", "0") == "1"
    if trace:
        _install_trace_shim()
    from concourse.bass_utils import run_bass_kernel_spmd
    import time as _time
    t0 = _time.time()
    res = run_bass_kernel_spmd(nc, in_maps, list(range(NCORES)), trace=trace)
    if os.environ.get("KGAT_VERBOSE", "0") == "1":
        print(f"launch wall {_time.time() - t0:.2f}s exec_ns={res.exec_time_ns}")
    if res.exec_time_ns is not None:
        LAST_EXEC_TIMES.append(res.exec_time_ns)
    return res.results


def _get_program(which, meta):
    key = (which, BF16, meta["epad"], tuple(meta["tchunks"]), meta.get("ngk"))
    if key not in _PROGRAMS:
        _PROGRAMS[key] = (_build_launch1 if which == 1 else _build_launch2)(meta)
    return _PROGRAMS[key]


def kernel(**inputs):
    global LAST_EXEC_TIMES
    LAST_EXEC_TIMES = []
    sim = os.environ.get("KGAT_SIM", "0") == "1"
    sim_cores = list(range(NCORES)) if sim else None

    x = np.asarray(inputs["x"], np.float32)
    ei = np.asarray(inputs["edge_index"], np.int64)
    batch = np.asarray(inputs["batch"], np.int64)
    edge_attr = np.asarray(inputs["edge_attr"], np.float32)

    W1ext, we1 = _prep_params(inputs["W1"], inputs["att_src1"], inputs["att_dst1"],
                              inputs["att_edge1"], inputs["We1"])
    W2ext, we2 = _prep_params(inputs["W2"], inputs["att_src2"], inputs["att_dst2"],
                              inputs["att_edge2"], inputs["We2"])
    b1 = np.asarray(inputs["b1"], np.float32)
    b2 = np.asarray(inputs["b2"], np.float32)
    lin_w = np.asarray(inputs["lin_w"], np.float32)
    lin_b = np.asarray(inputs["lin_b"], np.float32)

    cores, tchunks, epad = _prep_edges(ei[0], ei[1], edge_attr)
    pmats, g_lo, ngk = _prep_pool(batch)
    nchunk = epad // DT

    xpad = np.zeros((NPAD, F), np.float32)
    xpad[:N] = x
    xtc = np.ascontiguousarray(xpad.reshape(NCH_A, 128, F).transpose(0, 2, 1))
    iota8 = np.ascontiguousarray(
        np.tile(np.arange(128, dtype=np.float32), (128, BS)))
    iotap = np.arange(128, dtype=np.float32).reshape(128, 1).copy()

    meta = dict(epad=epad, nchunk=nchunk, tchunks=tchunks, ngk=ngk)

    def common_maps(k):
        ck = cores[k]
        return dict(esrc_w=ck["esrc_w"], hxidx_w=ck["hxidx_w"],
                    dstloc=ck["dstloc"], dstlocT=ck["dstlocT"],
                    eattrT=ck["eattrT"], iota8=iota8, iotap=iotap)

    nc1 = _get_program(1, meta)
    in_maps1 = []
    for k in range(NCORES):
        m = common_maps(k)
        m.update(xtc=xtc, W1ext=W1ext, W2ext=W2ext, we=we1, b=b1)
        in_maps1.append(m)
    res1 = _run(nc1, in_maps1, sim_cores=sim_cores)

    h2full = np.concatenate([res1[k]["h2slice"][:NS] for k in range(NCORES)], 0)
    h2pad = np.zeros((NPAD, TBW), h2full.dtype)
    h2pad[:N] = h2full

    nc2 = _get_program(2, meta)
    in_maps2 = []
    for k in range(NCORES):
        m = common_maps(k)
        m.update(h2ext=h2pad, pmat=pmats[k], we=we2, b=b2,
                 lin_w=lin_w, lin_b=lin_b)
        in_maps2.append(m)
    res2 = _run(nc2, in_maps2, sim_cores=sim_cores)

    y = np.zeros((G, 2), np.float32)
    for k in range(NCORES):
        yk = np.asarray(res2[k]["yout"], np.float32)
        for gi in range(ngk):
            g = g_lo[k] + gi
            if g < G:
                y[g] += yk[gi]
    return (y + lin_b[None, :]).astype(np.float32)
